# revision 1
# baseline (speedup 1.0000x reference)
"""Trainium2 kernel for nn_GCNRegression: linear-GCN scalar collapse.

The model is linear (no activation), so 4 GCN layers + mean-pool +
linear head collapse exactly to scalar propagation through the graph:
    c0 = W1 @ W2 @ W3 @ W4 @ Wl;  s0 = x @ c0
    s_k = dinv * (Adj @ (dinv * s_{k-1})) + b_k . c_k
    out[g] = sum_{v in g} s4[v] / n_max + bl
Runs on 8 NeuronCores: per-partition local_scatter routing + PE
transposes + PE segment-reduction (PSUM accumulate), AllGather between
rounds. All index arrays are host-precomputed from the edge list.
"""

import sys

sys.path.insert(0, "/opt/trn_rl_repo")

import numpy as np



P = 128          # partitions
SEGS = 16        # shard rows (psum partitions)
R4 = P // SEGS   # 4 rows per segment
NW = 3           # windows (= LS2/LS3 call count)
BPW = 6          # main blocks per window
BLKW = BPW + 1   # +1 ovf block per window
B_TOT = NW * BLKW  # total X2/XT blocks
CAP = R4 * BPW   # capacity per (p_s, w, s_v) cell


def cdiv(a, b):
    return (a + b - 1) // b


def _cumcount(keys):
    """Rank of each element within its key group (stable, array order)."""
    order = np.argsort(keys, kind="stable")
    sk = keys[order]
    grp_start = np.r_[0, np.flatnonzero(sk[1:] != sk[:-1]) + 1]
    sizes = np.diff(np.r_[grp_start, len(keys)])
    cum = np.arange(len(keys)) - np.repeat(grp_start, sizes)
    out = np.empty(len(keys), np.int64)
    out[order] = cum
    return out


def build_layout(n_nodes, nc):
    csh = cdiv(n_nodes, nc * SEGS)
    sh = SEGS * csh
    npad = nc * sh
    cf = npad // P
    return csh, sh, npad, cf


def relabel(edge_col_deg_src, n_nodes, nc):
    """edge_col_deg_src = deg array over original ids (len >= n_nodes).
    Shard by original id; within shard sort by in-degree desc; lay
    column-major into [SEGS, CSH]. Returns flat[] over padded ids."""
    deg = edge_col_deg_src
    csh, sh, npad, cf = build_layout(n_nodes, nc)
    flat = np.empty(npad, np.int64)
    for c in range(nc):
        ids = np.arange(c * sh, (c + 1) * sh)
        order = np.argsort(-deg[ids], kind="stable")
        t = np.empty(len(ids), np.int64)
        t[order] = np.arange(len(ids))
        s, cc = t % SEGS, t // SEGS
        flat[ids] = c * sh + s * csh + cc
    return flat, (csh, sh, npad, cf)


def build_core(core, re, ve, layout):
    """Per-core assignment. re/ve: device-flat src/dst positions."""
    csh, sh, npad, cf = layout
    E = len(re)
    p_s = re // cf
    fin = ve - core * sh
    s_v = fin // csh
    c_v = fin % csh

    # ---- window per source ----
    usrc, src_inv, src_cnt = np.unique(re, return_inverse=True, return_counts=True)
    usrc_p = usrc // cf
    so = np.lexsort((-src_cnt, usrc_p))
    rank_in_p = _cumcount(usrc_p[so])
    win_of_usrc = np.empty(len(usrc), np.int64)
    win_of_usrc[so] = rank_in_p % NW
    w_e = win_of_usrc[src_inv]

    # ---- overflow: cap (p_s, w, s_v) cells at CAP ----
    cell = (p_s * NW + w_e) * SEGS + s_v
    crank = _cumcount(cell)
    is_ovf = crank >= CAP
    # a source may contribute at most ~a few ovf edges; no per-source limit
    # needed (ovf path reads the expanded X stream).

    main = ~is_ovf
    # ---- j for main edges ----
    j_e = np.full(E, -1, np.int64)
    mi = np.flatnonzero(main)
    cnt_vw = _cumcount((ve[mi] * NW + w_e[mi]))
    j_e[mi] = s_v[mi] * R4 + (cnt_vw % R4)

    def psj(idx):
        return (p_s[idx] * NW + w_e[idx]) * P + j_e[idx]

    vwj = {}
    def vwj_key(i, jv):
        return (int(ve[i]) * NW + int(w_e[i])) * P + int(jv)
    for _try in range(300):
        k = psj(mi)
        cnt = np.bincount(k, minlength=P * NW * P)
        rank = _cumcount(k)
        move = np.flatnonzero(rank >= BPW)
        if len(move) == 0:
            break
        if _try == 0:
            vk = (ve[mi] * NW + w_e[mi]) * P + j_e[mi]
            uk, uc = np.unique(vk, return_counts=True)
            vwj = dict(zip(uk.tolist(), uc.tolist()))
        for ii in move:
            i = mi[ii]
            base = s_v[i] * R4
            pw = (p_s[i] * NW + w_e[i]) * P
            # candidates under BPW; pick min (v,w,j) count then min load
            best = None
            for r in range(R4):
                jv = base + r
                if jv == j_e[i]:
                    continue
                ld = cnt[pw + jv]
                nv = vwj.get(vwj_key(i, jv), 0)
                key = (nv, ld)
                if ld < BPW and (best is None or key < best[0]):
                    best = (key, jv)
            if best is None:
                # all full; pick min load anyway (will retry next sweep)
                loads = [cnt[pw + base + r] for r in range(R4)]
                jv = base + int(np.argmin(loads))
            else:
                jv = best[1]
            vwj[vwj_key(i, j_e[i])] = vwj.get(vwj_key(i, j_e[i]), 1) - 1
            cnt[pw + j_e[i]] -= 1
            j_e[i] = jv
            cnt[pw + jv] += 1
            vwj[vwj_key(i, jv)] = vwj.get(vwj_key(i, jv), 0) + 1
    else:
        raise RuntimeError("j balance failed")
    k = psj(mi)
    assert np.bincount(k, minlength=P * NW * P).max() <= BPW

    # ---- b for main ----
    b_e = np.full(E, -1, np.int64)
    b_e[mi] = w_e[mi] * BLKW + _cumcount(k)

    # ---- main layers: rank within (v, w, j) ----
    l_e = np.full(E, -1, np.int64)
    l_e[mi] = _cumcount((ve[mi] * NW + w_e[mi]) * P + j_e[mi])

    return dict(
        E=E, ve=ve, p_s=p_s, s_v=s_v, c_v=c_v, w_e=w_e, j_e=j_e, b_e=b_e,
        l_e=l_e, main=main, is_ovf_cap=is_ovf,
        usrc=usrc, usrc_p=usrc_p, usrc_q=usrc % cf, d_src=src_cnt,
        win_of_usrc=win_of_usrc, src_inv=src_inv,
    )


def assign_ovf(c, Lcap, rw):
    """Phase B: final overflow set = capacity spills + layer spills; assign
    ovf slots. Mutates c; returns nothing."""
    E = c["E"]
    r_v = c["c_v"] // rw
    spill_l = np.zeros(E, bool)
    mi = np.flatnonzero(c["main"])
    spill_l[mi] = c["l_e"][mi] >= Lcap[c["w_e"][mi], r_v[mi]]
    is_ovf = c["is_ovf_cap"] | spill_l
    c["main"] = ~is_ovf
    c["oi"] = oi = np.flatnonzero(is_ovf)
    ve, s_v, p_s, w_e = c["ve"], c["s_v"], c["p_s"], c["w_e"]
    c["jo"] = s_v[oi] * R4 + (_cumcount(ve[oi]) % R4)
    tcnt = _cumcount((p_s[oi] * NW + w_e[oi]))
    assert len(oi) == 0 or tcnt.max() < P, "ovf t overflow"
    c["t_o"] = (tcnt + p_s[oi] * 37 + w_e[oi] * 53) % P
    # g unique per (w, t, j*): per-window g-blocks
    c["g_o"] = _cumcount((w_e[oi] * P + c["t_o"]) * P + c["jo"])
    c["lo_"] = _cumcount(ve[oi] * P + c["jo"])


def finalize_cores(cores_raw, layout):
    csh, sh, npad, cf = layout
    nc = len(cores_raw)
    dmax = max(int(c["d_src"].max()) for c in cores_raw)

    # class sizes m[w][d]: max over (core, partition)
    m = np.zeros((NW, dmax + 1), np.int64)
    for c in cores_raw:
        cnt = np.zeros((P, NW, dmax + 1), np.int64)
        np.add.at(cnt, (c["usrc_p"], c["win_of_usrc"], c["d_src"]), 1)
        m = np.maximum(m, cnt.max(axis=0))
    m[:, 0] = 0

    x0_off = np.zeros((NW, dmax + 1), np.int64)
    x_off = np.zeros((NW, dmax + 1), np.int64)
    x0_woff = np.zeros(NW + 1, np.int64)
    x_woff = np.zeros(NW + 1, np.int64)
    o0 = o = 0
    expand_list = []
    for w in range(NW):
        x0_woff[w] = o0
        x_woff[w] = o
        for d in range(1, dmax + 1):
            if m[w][d] == 0:
                continue
            x0_off[w][d] = o0
            x_off[w][d] = o
            expand_list.append((int(o0), int(m[w][d]), d, int(o)))
            o0 += int(m[w][d])
            o += int(m[w][d]) * d
    x0_woff[NW] = o0
    x_woff[NW] = o
    CLS, XW = int(o0), int(o)
    assert 2 * CLS <= 2046, f"CLS={CLS}"

    B = B_TOT            # total X2 blocks (6 main + 1 ovf per window)
    F = B * P

    # main layer counts per (w, col-range); cap so each S window fits
    NRANGE = 16
    rw = cdiv(csh, NRANGE)
    widths = [min(rw, csh - r * rw) for r in range(NRANGE)]
    Lmax = np.zeros((NW, NRANGE), np.int64)
    for c in cores_raw:
        mm_ = c["main"]
        r_v = c["c_v"] // rw
        np.maximum.at(Lmax, (c["w_e"][mm_], r_v[mm_]), c["l_e"][mm_] + 1)
    SBUDGET = 1000
    wa = np.array(widths)
    for w in range(NW):
        while int((Lmax[w] * wa).sum()) > SBUDGET:
            r = int(np.argmax(Lmax[w] * 10000 + wa))
            assert Lmax[w][r] > 1, "cannot fit S window"
            Lmax[w][r] -= 1
    for c in cores_raw:
        assign_ovf(c, Lmax, rw)
    LOmax = np.zeros(NRANGE, np.int64)
    for c in cores_raw:
        r_v = c["c_v"] // rw
        if len(c["oi"]):
            np.maximum.at(LOmax, r_v[c["oi"]], c["lo_"] + 1)
    assert int((LOmax * wa).sum()) <= SBUDGET, f"ovf S window: {int((LOmax*wa).sum())}"

    G_w = np.ones(NW, np.int64)
    for c in cores_raw:
        if len(c["oi"]):
            np.maximum.at(G_w, c["w_e"][c["oi"]], c["g_o"] + 1)
    assert G_w.max() <= 7, f"G_w={G_w}"
    g_base = np.r_[0, np.cumsum(G_w)]
    G = int(g_base[-1])

    s_off = np.zeros((NW, NRANGE, int(Lmax.max() or 1)), np.int64)
    so_off = np.zeros((NRANGE, int(LOmax.max() or 1)), np.int64)
    s_woff = np.zeros(NW + 2, np.int64)
    so = 0
    mm_list = []
    for w in range(NW):
        s_woff[w] = so
        for r in range(NRANGE):
            for l in range(int(Lmax[w][r])):
                s_off[w][r][l] = so
                mm_list.append((int(so), int(widths[r]), int(r * rw)))
                so += int(widths[r])
    s_woff[NW] = so
    for r in range(NRANGE):
        for l in range(int(LOmax[r])):
            so_off[r][l] = so
            mm_list.append((int(so), int(widths[r]), int(r * rw)))
            so += int(widths[r])
    s_woff[NW + 1] = so
    SW = int(so)
    # split mm entries at psum bank boundaries (512 fp32 per bank)
    mm2 = []
    for (so_, wd, po) in mm_list:
        while wd > 0:
            room = 512 - (po % 512)
            take = min(wd, room)
            mm2.append((so_, take, po))
            so_ += take; po += take; wd -= take
    mm_list = mm2
    for w in range(NW + 1):
        assert 2 * (s_woff[w + 1] - s_woff[w]) <= 2046, f"S win {w} too wide"

    meta = dict(
        nc=nc, csh=csh, sh=sh, npad=npad, cf=cf, dmax=dmax,
        CLS=CLS, XW=XW, SW=SW, F=F, B=B, G=G, NRANGE=NRANGE, rw=rw,
        x0_off=x0_off, x_off=x_off, x0_woff=x0_woff, x_woff=x_woff,
        expand_list=expand_list, m=m, widths=widths,
        Lmax=Lmax, LOmax=LOmax, s_off=s_off, so_off=so_off, s_woff=s_woff,
        mm_list=mm_list, G_w=G_w, g_base=g_base,
    )
    per_core = [emit_core_arrays(c, meta) for c in cores_raw]
    return meta, per_core


def emit_core_arrays(c, meta):
    cf, csh = meta["cf"], meta["csh"]
    CLS, F, G = meta["CLS"], meta["F"], meta["G"]
    x0_off, x_off = meta["x0_off"], meta["x_off"]
    x_woff, s_woff = meta["x_woff"], meta["s_woff"]
    s_off, so_off = meta["s_off"], meta["so_off"]
    rw = meta["rw"]

    def put(arr, prt, pos, tgt):
        arr[prt, 2 * pos] = (2 * tgt).astype(np.int16)
        arr[prt, 2 * pos + 1] = (2 * tgt + 1).astype(np.int16)

    # class rank of each source within (p, w, d)
    cls_key = (c["usrc_p"] * NW + c["win_of_usrc"]) * (int(c["d_src"].max()) + 1) + c["d_src"]
    cls_rank = _cumcount(cls_key)

    # ls1
    ls1 = np.full((P, 2 * cf), -1, np.int16)
    tgt = x0_off[c["win_of_usrc"], c["d_src"]] + cls_rank
    assert tgt.max() < CLS
    put(ls1, c["usrc_p"], c["usrc_q"], tgt)

    # X position per edge
    r_in_src = _cumcount(c["src_inv"])
    si = c["src_inv"]
    xpos = x_off[c["w_e"], c["d_src"][si]] + cls_rank[si] * c["d_src"][si] + r_in_src

    ls2 = []
    for w in range(NW):
        wlen = int(x_woff[w + 1] - x_woff[w])
        a2 = np.full((P, 2 * wlen), -1, np.int16)
        selm = (c["w_e"] == w) & c["main"]
        xl = xpos[selm] - x_woff[w]
        t2 = (c["b_e"][selm] - w * BLKW) * P + c["j_e"][selm]
        put(a2, c["p_s"][selm], xl, t2)
        om = c["w_e"][c["oi"]] == w          # mask over oi order
        xo = xpos[c["oi"]][om] - x_woff[w]
        to = BPW * P + c["t_o"][om]
        put(a2, c["p_s"][c["oi"]][om], xo, to)
        ls2.append(a2)

    # ls3 (main): input XT[:, w*BLKW*128 : +BPW*128], partition j
    ls3 = []
    r_v = c["c_v"] // rw
    for w in range(NW):
        wlen = BPW * P
        slen = int(s_woff[w + 1] - s_woff[w])
        arr = np.full((P, 2 * wlen), -1, np.int16)
        selm = (c["w_e"] == w) & c["main"]
        ipos = (c["b_e"][selm] - w * BLKW) * P + c["p_s"][selm]
        t3 = (
            s_off[w, r_v[selm], c["l_e"][selm]]
            + (c["c_v"][selm] - r_v[selm] * rw)
            - s_woff[w]
        )
        assert len(t3) == 0 or (t3.min() >= 0 and t3.max() < slen)
        put(arr, c["j_e"][selm], ipos, t3)
        ls3.append(arr)

    # lsa call w: input XT ovf block (7w+6) [P, 128] -> XO chunk G_w blocks
    oi = c["oi"]
    lsa = []
    for w in range(NW):
        arr = np.full((P, 2 * P), -1, np.int16)
        if len(oi):
            sel = c["w_e"][oi] == w
            ipos = c["p_s"][oi][sel]
            ta = c["g_o"][sel] * P + c["jo"][sel]
            put(arr, c["t_o"][sel], ipos, ta)
        lsa.append(arr)

    # lsb: XOT [P, G*128] -> SM ovf window at (j*, so_off + col)
    g_base = meta["g_base"]
    slen_o = int(s_woff[NW + 1] - s_woff[NW])
    lsb = np.full((P, 2 * G * P), -1, np.int16)
    if len(oi):
        gg = g_base[c["w_e"][oi]] + c["g_o"]
        ipos = gg * P + c["t_o"]
        tb = (
            so_off[r_v[oi], c["lo_"]]
            + (c["c_v"][oi] - r_v[oi] * rw)
            - s_woff[NW]
        )
        assert tb.min() >= 0 and tb.max() < slen_o
        put(lsb, c["jo"], ipos, tb)

    return dict(ls1=ls1, ls2=ls2, ls3=ls3, lsa=lsa, lsb=lsb)


# ──────────────────────────────────────────────────────────────────────
# numpy emulation
# ──────────────────────────────────────────────────────────────────────

def _emu_ls(data_i16, idx_i16, num_elems_i16):
    Pp, n = idx_i16.shape
    assert data_i16.shape == (Pp, n)
    out = np.zeros((Pp, num_elems_i16), np.int16)
    for p in range(Pp):
        ii = idx_i16[p].astype(np.int64)
        valid = ii >= 0
        assert len(np.unique(ii[valid])) == valid.sum(), "dup idx"
        out[p, ii[valid]] = data_i16[p, valid]
    return out


def emulate_round(w_full, meta, arrs):
    cf, csh = meta["cf"], meta["csh"]
    CLS, XW, SW, F, B, G = (meta[k] for k in ("CLS", "XW", "SW", "F", "B", "G"))
    x_woff, s_woff = meta["x_woff"], meta["s_woff"]

    d16 = np.ascontiguousarray(w_full.astype(np.float32)).view(np.int16)
    x0 = _emu_ls(d16, arrs["ls1"], 2 * CLS).view(np.float32)

    x = np.zeros((P, XW), np.float32)
    for (o0, mm, d, o) in meta["expand_list"]:
        x[:, o : o + mm * d] = np.repeat(x0[:, o0 : o0 + mm], d, axis=1)

    x2 = np.zeros((P, F), np.float32)
    for w in range(NW):
        lo_, hi = int(x_woff[w]), int(x_woff[w + 1])
        seg = np.ascontiguousarray(x[:, lo_:hi]).view(np.int16)
        o = _emu_ls(seg, arrs["ls2"][w], 2 * BLKW * P).view(np.float32)
        x2[:, w * BLKW * P : (w + 1) * BLKW * P] = o

    xt = np.zeros((P, F), np.float32)
    for b in range(B):
        xt[:, b * P : (b + 1) * P] = x2[:, b * P : (b + 1) * P].T

    sm = np.zeros((P, SW), np.float32)
    for w in range(NW):
        sl = int(s_woff[w + 1] - s_woff[w])
        seg = np.ascontiguousarray(xt[:, w * BLKW * P : w * BLKW * P + BPW * P]).view(np.int16)
        o = _emu_ls(seg, arrs["ls3"][w], 2 * sl).view(np.float32)
        sm[:, int(s_woff[w]) : int(s_woff[w]) + sl] = o

    # ovf mini-pipeline
    xo = np.zeros((P, G * P), np.float32)
    g_base = meta["g_base"]
    for w in range(NW):
        seg = np.ascontiguousarray(xt[:, (w * BLKW + BPW) * P : (w * BLKW + BPW + 1) * P]).view(np.int16)
        gw = int(meta["G_w"][w])
        o = _emu_ls(seg, arrs["lsa"][w], 2 * gw * P).view(np.float32)
        xo[:, int(g_base[w]) * P : int(g_base[w] + gw) * P] = o
    xot = np.zeros((P, G * P), np.float32)
    for g in range(G):
        xot[:, g * P : (g + 1) * P] = xo[:, g * P : (g + 1) * P].T
    slo = int(s_woff[NW + 1] - s_woff[NW])
    if slo:
        seg = np.ascontiguousarray(xot).view(np.int16)
        o = _emu_ls(seg, arrs["lsb"], 2 * slo).view(np.float32)
        sm[:, int(s_woff[NW]) :] = o

    psum = np.zeros((SEGS, csh), np.float32)
    for (so, wd, po) in meta["mm_list"]:
        psum[:, po : po + wd] += sm[:, so : so + wd].reshape(SEGS, R4, wd).sum(axis=1)
    return psum


# ─── glue ───


class _H:
    pass


H = _H()
for _n in list(globals()):
    setattr(H, _n, globals()[_n])



def next_pow2(x):
    p = 1
    while p < x:
        p *= 2
    return p


def preprocess(x, edge_index, batch, nc_count=8, n_graphs=128):
    n_nodes = x.shape[0]
    row = np.asarray(edge_index[0], np.int64)
    col = np.asarray(edge_index[1], np.int64)
    batch = np.asarray(batch, np.int64)

    csh, sh, npad, cf = H.build_layout(n_nodes, nc_count)
    deg = np.bincount(col, minlength=npad).astype(np.int64)
    flat, layout = H.relabel(deg, n_nodes, nc_count)
    re, ve = flat[row], flat[col]

    cores_raw = []
    for c in range(nc_count):
        m = (ve // sh) == c
        cores_raw.append(H.build_core(c, re[m], ve[m], layout))
    meta, per_core = H.finalize_cores(cores_raw, layout)

    # device-order node arrays
    inv = np.empty(npad, np.int64)          # flat -> original id
    inv[flat] = np.arange(npad)
    deg_dev = deg[inv].astype(np.float32)   # deg at device flat position
    batch_dev = np.full(npad, -1, np.int64)
    batch_dev[flat[:n_nodes]] = batch[:n_nodes]

    # ---- pooling structures ----
    SEGS, P = H.SEGS, H.P
    g0 = np.zeros(nc_count, np.int64)
    ngl = np.zeros(nc_count, np.int64)
    wg_max = 0
    for c in range(nc_count):
        bd = batch_dev[c * sh:(c + 1) * sh]
        real = bd >= 0
        gmin, gmax = (int(bd[real].min()), int(bd[real].max())) if real.any() else (0, 0)
        g0[c], ngl[c] = gmin, gmax - gmin + 1
        # count per (row s, local g)
        fin = np.arange(sh)
        s = fin // csh
        cnt = np.zeros((SEGS, int(ngl[c])), np.int64)
        np.add.at(cnt, (s[real], bd[real] - gmin), 1)
        wg_max = max(wg_max, int(cnt.max()))
    NGLP = int(ngl.max())
    WGP = next_pow2(wg_max)
    GPH = max(1, min(1023 // WGP, NGLP))    # local graphs per pool call
    NPH = H.cdiv(NGLP, GPH)
    PHALF = GPH * WGP
    assert NPH * GPH <= 32, f"pool pad {NPH * GPH} > 32"

    pool_idx = []
    gms = [np.zeros((P, P), np.float32) for _ in range(2)]
    for c in range(nc_count):
        bd = batch_dev[c * sh:(c + 1) * sh]
        fin = np.arange(sh)
        s, cc = fin // csh, fin % csh
        lg = bd - g0[c]
        arrs = []
        rank = np.zeros(sh, np.int64)
        real = bd >= 0
        key = s * 4096 + lg
        rank[real] = H._cumcount(key[real])
        assert rank.max() < WGP
        for h in range(NPH):
            a = np.full((SEGS, 2 * csh), -1, np.int16)
            selh = real & (lg >= h * GPH) & (lg < (h + 1) * GPH)
            tgt = (lg[selh] - h * GPH) * WGP + rank[selh]
            assert len(tgt) == 0 or tgt.max() < PHALF
            a[s[selh], 2 * cc[selh]] = (2 * tgt).astype(np.int16)
            a[s[selh], 2 * cc[selh] + 1] = (2 * tgt + 1).astype(np.int16)
            arrs.append(a)
        pool_idx.append(arrs)
        # gm matrices: partall flat pos = c*32 + (h*GPH + glocal)
        for li in range(NPH * GPH):
            g = g0[c] + li
            if li < int(ngl[c]) and g < n_graphs:
                fp = c * 32 + li
                gms[fp % 2][fp // 2, g] = 1.0

    pool_meta = dict(NGLP=NPH * GPH, NGH=GPH, WGP=WGP, NPH=NPH, PHALF=PHALF,
                     g0=g0)
    return meta, per_core, pool_meta, pool_idx, gms, flat, deg_dev, layout


def make_inputs(meta, pool_meta, per_core, pool_idx, gms, flat, deg_dev,
                x, weights, n_max, n_graphs=128):
    """Build per-core in_maps. weights = dict(W1..Wl, b1..bl)."""
    csh, sh, npad, cf = meta["csh"], meta["sh"], meta["npad"], meta["cf"]
    nc_count = meta["nc"]
    n_nodes = x.shape[0]

    # x in device order, transposed: xT_dev[c] = [128, sh]
    xdev = np.zeros((npad, x.shape[1]), np.float32)
    xdev[flat[:n_nodes]] = x
    in_maps = []
    for c in range(nc_count):
        im = dict(
            xT=np.ascontiguousarray(xdev[c * sh:(c + 1) * sh].T),
            degf=deg_dev.reshape(H.P, cf),
            degs=deg_dev[c * sh:(c + 1) * sh].reshape(H.SEGS, csh),
            ls1=per_core[c]["ls1"],
            lsb=per_core[c]["lsb"],
            w1t=np.ascontiguousarray(weights["W1"].T),
            w2t=np.ascontiguousarray(weights["W2"].T),
            w3t=np.ascontiguousarray(weights["W3"].T),
            w4t=np.ascontiguousarray(weights["W4"].T),
            wl=np.ascontiguousarray(weights["Wl"]).reshape(64, 1),
            bl=np.asarray(weights["bl"], np.float32).reshape(1, 1),
            invn=np.asarray([[1.0 / np.float32(n_max)]], np.float32),
            sel=np.repeat(np.eye(H.SEGS, dtype=np.float32),
                          H.R4, axis=0),
            ident=np.eye(H.P, dtype=np.float32),
            ones16=np.ones((H.SEGS, 1), np.float32),
            gm0=gms[0], gm1=gms[1],
        )
        for k in range(1, 5):
            im[f"b{k}"] = np.asarray(weights[f"b{k}"], np.float32).reshape(64, 1)
        for w in range(H.NW):
            im[f"ls2_{w}"] = per_core[c]["ls2"][w]
            im[f"ls3_{w}"] = per_core[c]["ls3"][w]
            im[f"lsa_{w}"] = per_core[c]["lsa"][w]
        for h in range(pool_meta["NPH"]):
            im[f"pool_{h}"] = pool_idx[c][h]
        in_maps.append(im)
    return in_maps


def reference_numpy(x, edge_index, batch, weights, n_graphs=128):
    """Direct numpy reference of the original model."""
    row = np.asarray(edge_index[0]); col = np.asarray(edge_index[1])
    N = x.shape[0]; E = row.shape[0]
    deg = np.bincount(col, minlength=N).astype(np.float64)
    dinv = np.where(deg > 0, deg ** -0.5, 0.0)
    norm = dinv[row] * dinv[col]
    h = x.astype(np.float64)
    for k in range(1, 5):
        W = weights[f"W{k}"] if k > 1 else weights["W1"]
        b = weights[f"b{k}"]
        hw = h @ W
        msg = norm[:, None] * hw[row]
        out = np.zeros((N, hw.shape[1]))
        np.add.at(out, col, msg)
        h = out + b
    sums = np.zeros((n_graphs, h.shape[1]))
    np.add.at(sums, batch, h)
    counts = np.bincount(batch, minlength=n_graphs)
    pooled = sums / counts.max()
    return (pooled @ weights["Wl"] + weights["bl"]).astype(np.float32)


# ─── device kernel ───
from contextlib import ExitStack

import concourse.bass as bass
import concourse.tile as tile
from concourse import bacc, mybir

FP32 = mybir.dt.float32
I16 = mybir.dt.int16
AT = mybir.ActivationFunctionType
OP = mybir.AluOpType




def build_kernel(meta, pool_meta, n_graphs=128):
    P, SEGS, NW, BPW, BLKW = H.P, H.SEGS, H.NW, H.BPW, H.BLKW
    csh, sh, npad, cf = meta["csh"], meta["sh"], meta["npad"], meta["cf"]
    CLS, XW, SW, F, B, G = (meta[k] for k in ("CLS", "XW", "SW", "F", "B", "G"))
    x0_woff, x_woff, s_woff = meta["x0_woff"], meta["x_woff"], meta["s_woff"]
    G_w, g_base = meta["G_w"], meta["g_base"]
    NGLP, WGP = pool_meta["NGLP"], pool_meta["WGP"]
    PHALF = pool_meta["PHALF"]          # pool output fp32 per call
    NPH = pool_meta["NPH"]
    core_ids = list(range(meta["nc"]))

    nc = bacc.Bacc("TRN2", target_bir_lowering=False, debug=False,
                   num_devices=meta["nc"])

    def din(name, shape, dt=FP32):
        return nc.declare_dram_parameter(name, list(shape), dt, isOutput=False)

    # ---- inputs ----
    xT_in = din("xT", [P, sh])
    degf_in = din("degf", [P, cf])
    degs_in = din("degs", [SEGS, csh])
    ls1_in = din("ls1", [P, 2 * cf], I16)
    ls2_in = [din(f"ls2_{w}", [P, 2 * int(x_woff[w + 1] - x_woff[w])], I16)
              for w in range(NW)]
    ls3_in = [din(f"ls3_{w}", [P, 2 * BPW * P], I16) for w in range(NW)]
    lsa_in = [din(f"lsa_{w}", [P, 2 * P], I16) for w in range(NW)]
    lsb_in = din("lsb", [P, 2 * G * P], I16)
    pool_in = [din(f"pool_{h}", [SEGS, 2 * csh], I16) for h in range(NPH)]
    w1t_in = din("w1t", [64, 128])
    w2t_in = din("w2t", [64, 64])
    w3t_in = din("w3t", [64, 64])
    w4t_in = din("w4t", [64, 64])
    wl_in = din("wl", [64, 1])
    b_in = [din(f"b{k}", [64, 1]) for k in range(1, 5)]
    bl_in = din("bl", [1, 1])
    invn_in = din("invn", [1, 1])
    sel_in = din("sel", [P, SEGS])
    ident_in = din("ident", [P, P])
    ones16_in = din("ones16", [SEGS, 1])
    gm_in = [din(f"gm{i}", [P, P]) for i in range(2)]
    out_ext = nc.declare_dram_parameter("out", [n_graphs], FP32, isOutput=True)

    # ---- internal DRAM ----
    sh_dram = nc.dram_tensor("sh_dram", [sh], FP32)
    full_dram = nc.dram_tensor("full_dram", [npad], FP32, addr_space="Shared")
    part_dram = nc.dram_tensor("part_dram", [32], FP32)
    warm_in = nc.dram_tensor("warm_in", [32], FP32)
    warm_out = nc.dram_tensor("warm_out", [256], FP32, addr_space="Shared")
    partall_dram = nc.dram_tensor("partall_dram", [256], FP32, addr_space="Shared")

    with tile.TileContext(nc) as tc:
        with ExitStack() as ctx:
            pool = ctx.enter_context(tc.tile_pool(name="p", bufs=1))
            tp = ctx.enter_context(tc.tile_pool(name="tp", bufs=2, space="PSUM"))
            up = ctx.enter_context(tc.tile_pool(name="up", bufs=1, space="PSUM"))

            # persistent tiles
            state = pool.tile([P, cf], FP32)
            wbuf = pool.tile([P, cf], FP32)
            x0 = pool.tile([P, CLS], FP32)
            xbuf = pool.tile([P, XW], FP32)
            x2 = pool.tile([P, F], FP32)
            xt = pool.tile([P, F], FP32)
            sm = pool.tile([P, SW], FP32)
            xo = pool.tile([P, G * P], FP32)
            xot = pool.tile([P, G * P], FP32)
            dinvf = pool.tile([P, cf], FP32)
            dinvs = pool.tile([SEGS, csh], FP32)
            u_sb = pool.tile([SEGS, csh], FP32)
            s_sb = pool.tile([SEGS, csh], FP32)
            sel = pool.tile([P, SEGS], FP32)
            ident = pool.tile([P, P], FP32)
            ones16 = pool.tile([SEGS, 1], FP32)
            gm = [pool.tile([P, P], FP32, name=f"gm{i}") for i in range(2)]
            ls1 = pool.tile([P, 2 * cf], I16)
            ls2 = [pool.tile([P, 2 * int(x_woff[w + 1] - x_woff[w])], I16,
                             name=f"ls2t{w}") for w in range(NW)]
            ls3 = [pool.tile([P, 2 * BPW * P], I16, name=f"ls3t{w}") for w in range(NW)]
            lsa = [pool.tile([P, 2 * P], I16, name=f"lsat{w}") for w in range(NW)]
            lsb = pool.tile([P, 2 * G * P], I16)
            plidx = [pool.tile([SEGS, 2 * csh], I16, name=f"plidx{h}") for h in range(NPH)]
            poolbuf = pool.tile([SEGS, NPH * PHALF], FP32)
            p16 = pool.tile([SEGS, NGLP], FP32)
            part_sb = pool.tile([1, 32], FP32)
            partall = pool.tile([P, 2], FP32)
            outrow = pool.tile([1, n_graphs], FP32)
            wts = {
                "w1t": pool.tile([64, 128], FP32, name="w1t_t"),
                "w2t": pool.tile([64, 64], FP32, name="w2t_t"),
                "w3t": pool.tile([64, 64], FP32, name="w3t_t"),
                "w4t": pool.tile([64, 64], FP32, name="w4t_t"),
                "wl": pool.tile([64, 1], FP32, name="wl_t"),
            }
            bs = [pool.tile([64, 1], FP32, name=f"bs{k}") for k in range(4)]
            blt = pool.tile([1, 1], FP32)
            invn = pool.tile([1, 1], FP32)
            cvec = {
                "c3": pool.tile([64, 1], FP32, name="c3t"),
                "c2": pool.tile([64, 1], FP32, name="c2t"),
                "c1": pool.tile([64, 1], FP32, name="c1t"),
                "c0": pool.tile([128, 1], FP32, name="c0t"),
            }
            betas = pool.tile([1, 4], FP32)
            degf = state
            degs = s_sb

            # ---- loads ----

            warmsb = pool.tile([1, 32], FP32)
            nc.vector.memset(warmsb[:], 0.0)
            nc.sync.dma_start(warm_in[:].rearrange("(a b) -> a b", a=1),
                              warmsb[:])
            nc.gpsimd.collective_compute(
                "AllGather", OP.bypass, replica_groups=[core_ids],
                ins=[warm_in[:]], outs=[warm_out[:]],
            )
            nc.sync.dma_start(ls1[:], ls1_in[:])
            for w in range(NW):
                nc.sync.dma_start(ls2[w][:], ls2_in[w][:])
                nc.sync.dma_start(ls3[w][:], ls3_in[w][:])
                nc.sync.dma_start(lsa[w][:], lsa_in[w][:])
            nc.sync.dma_start(lsb[:], lsb_in[:])
            for h in range(NPH):
                nc.sync.dma_start(plidx[h][:], pool_in[h][:])
            nc.sync.dma_start(sel[:], sel_in[:])
            nc.sync.dma_start(ident[:], ident_in[:])
            nc.sync.dma_start(ones16[:], ones16_in[:])
            for i in range(2):
                nc.sync.dma_start(gm[i][:], gm_in[i][:])
            for k, t in wts.items():
                nc.sync.dma_start(t[:], {"w1t": w1t_in, "w2t": w2t_in,
                                         "w3t": w3t_in, "w4t": w4t_in,
                                         "wl": wl_in}[k][:])
            for k in range(4):
                nc.sync.dma_start(bs[k][:], b_in[k][:])
            nc.sync.dma_start(blt[:], bl_in[:])
            nc.sync.dma_start(invn[:], invn_in[:])
            nc.sync.dma_start(degf[:], degf_in[:])
            nc.sync.dma_start(degs[:], degs_in[:])

            # ---- dinv = rsqrt(deg + (deg==0)) * (deg>0) ----
            def make_dinv(dst, deg_t, tmp, shape):
                nc.vector.tensor_scalar(tmp[:], deg_t[:], 0.0, None, OP.is_equal)
                nc.vector.tensor_tensor(tmp[:], tmp[:], deg_t[:], OP.add)
                nc.scalar.activation(tmp[:], tmp[:], AT.Sqrt)
                nc.vector.reciprocal(tmp[:], tmp[:])
                nc.vector.tensor_scalar(dst[:], deg_t[:], 0.0, None, OP.is_gt)
                nc.vector.tensor_tensor(dst[:], dst[:], tmp[:], OP.mult)

            make_dinv(dinvf, degf, wbuf, (P, cf))
            make_dinv(dinvs, degs, u_sb[0:SEGS, :], (SEGS, csh))

            # ---- c chain + betas ----
            pc = tp.tile([128, 4], FP32, tag="ops", bufs=1)
            nc.tensor.matmul(pc[0:64, 0:1], wts["w4t"][:], wts["wl"][:],
                             start=True, stop=True)
            nc.vector.tensor_copy(cvec["c3"][:], pc[0:64, 0:1])
            nc.tensor.matmul(pc[0:64, 1:2], wts["w3t"][:], cvec["c3"][:],
                             start=True, stop=True)
            nc.vector.tensor_copy(cvec["c2"][:], pc[0:64, 1:2])
            nc.tensor.matmul(pc[0:64, 2:3], wts["w2t"][:], cvec["c2"][:],
                             start=True, stop=True)
            nc.vector.tensor_copy(cvec["c1"][:], pc[0:64, 2:3])
            nc.tensor.matmul(pc[0:128, 3:4], wts["w1t"][:], cvec["c1"][:],
                             start=True, stop=True)
            nc.vector.tensor_copy(cvec["c0"][:], pc[0:128, 3:4])
            pb = tp.tile([1, 4], FP32, tag="ops", bufs=1)
            for k, cn in enumerate(["c1", "c2", "c3"]):
                nc.tensor.matmul(pb[0:1, k:k + 1], bs[k][:], cvec[cn][:],
                                 start=True, stop=True)
            nc.tensor.matmul(pb[0:1, 3:4], bs[3][:], wts["wl"][:],
                             start=True, stop=True)
            nc.vector.tensor_copy(betas[:], pb[:])
            # broadcast betas to 16 partitions: [16, 4]
            ones116 = pool.tile([1, 16], FP32)
            betas16 = pool.tile([SEGS, 4], FP32)
            nc.vector.memset(ones116[:], 1.0)
            pbb = tp.tile([SEGS, 4], FP32, tag="ops", bufs=1)
            nc.tensor.matmul(pbb[:], ones116[:], betas[:], start=True, stop=True)
            nc.vector.tensor_copy(betas16[:], pbb[:])

            # ---- s0 = x @ c0 (per shard, via xT chunks) ----
            stage = pool.tile([1, sh], FP32)
            hf = csh // 2
            s0_cuts = [(0, hf), (hf, csh)]
            SPQ = SEGS // 4
            for q in range(4):
                xq = pool.tile([P, SPQ * csh], FP32, tag="xq", bufs=2,
                               name=f"xq{q}")
                nc.sync.dma_start(
                    xq[:], xT_in[:, q * SPQ * csh:(q + 1) * SPQ * csh])
                for sl in range(SPQ):
                    s = q * SPQ + sl
                    for ci, (a, b2) in enumerate(s0_cuts):
                        ps0 = tp.tile([1, hf + 1], FP32, tag="ps0", bufs=2,
                                      name=f"ps0_{s}_{ci}")
                        nc.tensor.matmul(ps0[0:1, 0:b2 - a], cvec["c0"][:],
                                         xq[:, sl * csh + a:sl * csh + b2],
                                         start=True, stop=True)
                        nc.vector.tensor_copy(
                            stage[:, s * csh + a:s * csh + b2],
                            ps0[0:1, 0:b2 - a])

            # state rounds
            def allgather_state(src_ap):
                nc.sync.dma_start(
                    sh_dram[:].rearrange("(a b) -> a b", a=src_ap.shape[0]),
                    src_ap)
                nc.gpsimd.collective_compute(
                    "AllGather", OP.bypass, replica_groups=[core_ids],
                    ins=[sh_dram[:]], outs=[full_dram[:]],
                )
                nc.sync.dma_start(
                    state[:], full_dram[:].rearrange("(p c) -> p c", p=P))

            for rnd in range(4):
                allgather_state(stage[:] if rnd == 0 else s_sb[:])
                # w = state * dinv
                nc.vector.tensor_tensor(wbuf[:], state[:], dinvf[:], OP.mult)
                # LS1
                nc.gpsimd.local_scatter(
                    x0[:].bitcast(I16), wbuf[:].bitcast(I16), ls1[:],
                    channels=P, num_elems=2 * CLS, num_idxs=2 * cf)
                # expand
                for (o0, mm_, d, o) in meta["expand_list"]:
                    src = x0[:, o0:o0 + mm_].unsqueeze(2).broadcast_to([P, mm_, d])
                    nc.vector.tensor_copy(
                        xbuf[:, o:o + mm_ * d].rearrange("p (m d) -> p m d", d=d),
                        src)
                # LS2 (+ ovf block)
                for w in range(NW):
                    lo_, hi = int(x_woff[w]), int(x_woff[w + 1])
                    nc.gpsimd.local_scatter(
                        x2[:, w * BLKW * P:(w + 1) * BLKW * P].bitcast(I16),
                        xbuf[:, lo_:hi].bitcast(I16), ls2[w][:],
                        channels=P, num_elems=2 * BLKW * P,
                        num_idxs=2 * (hi - lo_))
                # transposes (4 blocks per psum bank, 1 copy per bank)
                for b0 in range(0, B, 4):
                    nb = min(4, B - b0)
                    pt = tp.tile([P, 4 * P], FP32, tag="ptr", name=f"pt{b0}")
                    for k in range(nb):
                        b = b0 + k
                        nc.tensor.transpose(pt[:, k * P:(k + 1) * P],
                                            x2[:, b * P:(b + 1) * P], ident[:])
                    nc.vector.tensor_copy(xt[:, b0 * P:(b0 + nb) * P],
                                          pt[:, 0:nb * P])
                # LS3 main
                for w in range(NW):
                    sl = int(s_woff[w + 1] - s_woff[w])
                    nc.gpsimd.local_scatter(
                        sm[:, int(s_woff[w]):int(s_woff[w]) + sl].bitcast(I16),
                        xt[:, w * BLKW * P: w * BLKW * P + BPW * P].bitcast(I16),
                        ls3[w][:], channels=P, num_elems=2 * sl,
                        num_idxs=2 * BPW * P)
                # LSA
                slo = int(s_woff[NW + 1] - s_woff[NW])
                for w in range(NW if slo > 0 else 0):
                    gw = int(G_w[w])
                    gb = int(g_base[w])
                    nc.gpsimd.local_scatter(
                        xo[:, gb * P:(gb + gw) * P].bitcast(I16),
                        xt[:, (w * BLKW + BPW) * P:(w * BLKW + BPW + 1) * P].bitcast(I16),
                        lsa[w][:], channels=P, num_elems=2 * gw * P,
                        num_idxs=2 * P)
                # XO transposes
                for g0 in range(0, G if slo > 0 else 0, 4):
                    ng = min(4, G - g0)
                    pt = tp.tile([P, 4 * P], FP32, tag="ptr", name=f"po{g0}")
                    for k in range(ng):
                        g = g0 + k
                        nc.tensor.transpose(pt[:, k * P:(k + 1) * P],
                                            xo[:, g * P:(g + 1) * P], ident[:])
                    nc.vector.tensor_copy(xot[:, g0 * P:(g0 + ng) * P],
                                          pt[:, 0:ng * P])
                # LSB
                if slo > 0:
                    nc.gpsimd.local_scatter(
                        sm[:, int(s_woff[NW]):int(s_woff[NW]) + slo].bitcast(I16),
                        xot[:].bitcast(I16), lsb[:],
                        channels=P, num_elems=2 * slo, num_idxs=2 * G * P)
                # sel matmuls, grouped by psum col-range
                pu = up.tile([SEGS, csh], FP32, tag="psum_u")
                by_range = {}
                for (so, wd, po) in meta["mm_list"]:
                    by_range.setdefault((po, wd), []).append(so)
                for (po, wd), sos in by_range.items():
                    for i, so in enumerate(sos):
                        nc.tensor.matmul(
                            pu[:, po:po + wd], sel[:], sm[:, so:so + wd],
                            start=(i == 0), stop=(i == len(sos) - 1))
                nc.vector.tensor_copy(u_sb[:], pu[:])
                # s' = u * dinvs + beta_k
                nc.vector.tensor_tensor(s_sb[:], u_sb[:], dinvs[:], OP.mult)
                nc.vector.tensor_scalar(
                    s_sb[:], s_sb[:], betas16[:, rnd:rnd + 1], None, OP.add)

            # ---- pooling ----
            for h in range(NPH):
                nc.gpsimd.local_scatter(
                    poolbuf[:, h * PHALF:(h + 1) * PHALF].bitcast(I16),
                    s_sb[:].bitcast(I16), plidx[h][:],
                    channels=SEGS, num_elems=2 * PHALF, num_idxs=2 * csh)
            # tree reduce over WGP (pool slots per (row, local graph))
            wgp = WGP
            a = poolbuf[:].rearrange("s (g t) -> s g t", t=WGP)
            while wgp > 1:
                hw = wgp // 2
                nc.vector.tensor_tensor(
                    a[:, :, 0:hw], a[:, :, 0:hw], a[:, :, hw:wgp], OP.add)
                wgp = hw
            nc.vector.tensor_copy(p16[:], a[:, :, 0:1].rearrange("s g t -> s (g t)"))
            pp = tp.tile([1, NGLP], FP32, tag="ops", bufs=1)
            nc.tensor.matmul(pp[:], ones16[:], p16[:], start=True, stop=True)
            nc.vector.memset(part_sb[:], 0.0)
            nc.vector.tensor_copy(part_sb[:, 0:NGLP], pp[:])
            nc.sync.dma_start(part_dram[:].rearrange("(a b) -> a b", a=1),
                              part_sb[:])
            nc.gpsimd.collective_compute(
                "AllGather", OP.bypass, replica_groups=[core_ids],
                ins=[part_dram[:]], outs=[partall_dram[:]],
            )
            nc.sync.dma_start(partall[:],
                              partall_dram[:].rearrange("(p c) -> p c", p=P))
            po_ = tp.tile([1, n_graphs], FP32, tag="ops", bufs=1)
            nc.tensor.matmul(po_[:], partall[:, 0:1], gm[0][:],
                             start=True, stop=False)
            nc.tensor.matmul(po_[:], partall[:, 1:2], gm[1][:],
                             start=False, stop=True)
            nc.vector.tensor_copy(outrow[:], po_[:])
            nc.vector.tensor_scalar(outrow[:], outrow[:], invn[0:1, 0:1],
                                    None, OP.mult)
            nc.vector.tensor_scalar(outrow[:], outrow[:], blt[0:1, 0:1],
                                    None, OP.add)
            nc.sync.dma_start(out_ext[:].rearrange("(a b) -> a b", a=1),
                              outrow[:])
    return nc


# ─── entry point ───

def kernel(x, edge_index, batch, W1, b1, W2, b2, W3, b3, W4, b4, Wl, bl):
    from concourse.bass_utils import run_bass_kernel_spmd

    x = np.asarray(x, np.float32)
    edge_index = np.asarray(edge_index)
    batch = np.asarray(batch)
    weights = dict(W1=np.asarray(W1, np.float32), W2=np.asarray(W2, np.float32),
                   W3=np.asarray(W3, np.float32), W4=np.asarray(W4, np.float32),
                   Wl=np.asarray(Wl, np.float32),
                   b1=np.asarray(b1, np.float32), b2=np.asarray(b2, np.float32),
                   b3=np.asarray(b3, np.float32), b4=np.asarray(b4, np.float32),
                   bl=np.asarray(bl, np.float32))
    n_graphs = 128

    meta, per_core, pool_meta, pool_idx, gms, flat, deg_dev, layout = \
        preprocess(x, edge_index, batch, 8, n_graphs)
    n_max = int(np.bincount(np.asarray(batch, np.int64),
                            minlength=n_graphs).max())
    in_maps = make_inputs(meta, pool_meta, per_core, pool_idx, gms, flat,
                          deg_dev, x, weights, n_max, n_graphs)
    nc = build_kernel(meta, pool_meta, n_graphs)
    nc.finalize()
    res = run_bass_kernel_spmd(nc, in_maps, core_ids=list(range(8)),
                               trace=False)
    return res.results[0]["out"].reshape(n_graphs, 1).astype(np.float32)



# revision 9
# speedup vs baseline: 1.4455x; 1.4455x over previous
"""Trainium2 kernel for nn_GCNRegression: linear-GCN scalar collapse, bf16.

The model is linear (no activation), so 4 GCN layers + mean-pool + linear
head collapse exactly to scalar propagation through the graph:
    c0 = W1 @ W2 @ W3 @ W4 @ Wl;  s0 = x @ c0
    s_k = dinv * (Adj^T @ (dinv * s_{k-1})) + b_k . c_k
    out[g] = sum_{v in g} s4[v] / n_max + bl
8 NeuronCores, destination-sharded edges. Per round: AllGather the
dinv-pre-scaled bf16 state, per-partition local_scatter routing (gather
classes -> expand by out-degree -> scatter into transpose blocks), PE
transposes, local_scatter into a layer-major segment layout, DVE layer
sums, one PSUM-accumulated sel matmul, scale + bias.  All index arrays
are host-precomputed from the edge list; values move as bf16 (single
int16 slot per value in every local_scatter).
"""

import sys

sys.path.insert(0, "/opt/trn_rl_repo")

import numpy as np
import ml_dtypes

BF16 = ml_dtypes.bfloat16

P = 128          # partitions
SEGS = 16        # shard rows (psum partitions)
R4 = P // SEGS   # 8 partition rows per segment
NW = 3           # windows (= LS2/LS3 call count)
NRANGE = 16      # column ranges for layer caps
MAXELEMS = 2046  # local_scatter out-region limit (num_elems*32 < 2^16)


def cdiv(a, b):
    return (a + b - 1) // b


def even(x):
    return x + (x & 1)


def _cumcount(keys):
    """Rank of each element within its key group (stable, array order)."""
    order = np.argsort(keys, kind="stable")
    sk = keys[order]
    grp_start = np.r_[0, np.flatnonzero(sk[1:] != sk[:-1]) + 1]
    sizes = np.diff(np.r_[grp_start, len(keys)])
    cum = np.arange(len(keys)) - np.repeat(grp_start, sizes)
    out = np.empty(len(keys), np.int64)
    out[order] = cum
    return out


def build_layout(n_nodes, nc):
    csh = cdiv(n_nodes, nc * SEGS)
    sh = SEGS * csh
    npad = nc * sh
    cf = npad // P
    return csh, sh, npad, cf


def relabel(deg, n_nodes, nc):
    """Shard by original id; within shard sort by in-degree desc; lay
    column-major into [SEGS, CSH]. Returns flat[] over padded ids."""
    csh, sh, npad, cf = build_layout(n_nodes, nc)
    flat = np.empty(npad, np.int64)
    for c in range(nc):
        ids = np.arange(c * sh, (c + 1) * sh)
        order = np.argsort(-deg[ids], kind="stable")
        t = np.empty(len(ids), np.int64)
        t[order] = np.arange(len(ids))
        s, cc = t % SEGS, t // SEGS
        flat[ids] = c * sh + s * csh + cc
    return flat, (csh, sh, npad, cf)


def build_core(core, re, ve, layout, bpw):
    """Per-core routing. re/ve: device-flat src/dst positions."""
    csh, sh, npad, cf = layout
    E = len(re)
    p_s = re // cf
    q_s = re % cf
    fin = ve - core * sh
    s_v = fin // csh
    c_v = fin % csh

    # ---- window per source: per (partition, count-desc) round robin ----
    usrc, src_inv, src_cnt = np.unique(re, return_inverse=True,
                                       return_counts=True)
    usrc_p = usrc // cf
    so = np.lexsort((-src_cnt, usrc_p))
    rank_in_p = _cumcount(usrc_p[so])
    win_of_usrc = np.empty(len(usrc), np.int64)
    win_of_usrc[so] = rank_in_p % NW
    w_e = win_of_usrc[src_inv]

    # ---- initial j: balanced round-robin within (v, w) ----
    jr = _cumcount(fin * NW + w_e) % R4
    j_e = s_v * R4 + jr

    # ---- repair (p_s, w, j) column loads to <= bpw ----
    def psj(j):
        return (p_s * NW + w_e) * P + j

    cnt = np.bincount(psj(j_e), minlength=P * NW * P)
    vwj_key_all = (fin * NW + w_e) * P
    vwj = {}
    vk = vwj_key_all + j_e
    uk, uc = np.unique(vk, return_counts=True)
    vwj = dict(zip(uk.tolist(), uc.tolist()))
    for _try in range(400):
        rank = _cumcount(psj(j_e))
        move = np.flatnonzero(rank >= bpw)
        if len(move) == 0:
            break
        for i in move:
            base = s_v[i] * R4
            pw = (p_s[i] * NW + w_e[i]) * P
            best = None
            for r in range(R4):
                jv = base + r
                if jv == j_e[i]:
                    continue
                ld = cnt[pw + jv]
                if ld >= bpw:
                    continue
                nv = vwj.get(vwj_key_all[i] + jv, 0)
                key = (nv, ld)
                if best is None or key < best[0]:
                    best = (key, jv)
            if best is None:
                ld0 = [cnt[pw + base + r] for r in range(R4)]
                jv = base + int(np.argmin(ld0))
            else:
                jv = best[1]
            vwj[vwj_key_all[i] + j_e[i]] -= 1
            cnt[pw + j_e[i]] -= 1
            j_e[i] = jv
            cnt[pw + jv] += 1
            vwj[vwj_key_all[i] + jv] = vwj.get(vwj_key_all[i] + jv, 0) + 1
    else:
        raise RuntimeError("j balance failed")
    assert np.bincount(psj(j_e), minlength=P * NW * P).max() <= bpw

    b_e = _cumcount(psj(j_e))                       # block rank in [0,bpw)
    l_e = _cumcount((fin * NW + w_e) * P + j_e)     # layer rank per (v,w,j)
    r_in_src = _cumcount(src_inv)                   # edge rank within source

    return dict(
        E=E, p_s=p_s, q_s=q_s, s_v=s_v, c_v=c_v, w_e=w_e, j_e=j_e,
        b_e=b_e, l_e=l_e, r_in_src=r_in_src,
        usrc=usrc, usrc_p=usrc_p, usrc_q=usrc % cf, d_src=src_cnt,
        win_of_usrc=win_of_usrc, src_inv=src_inv,
    )


def finalize_cores(cores_raw, layout, bpw):
    csh, sh, npad, cf = layout
    nc = len(cores_raw)
    dmax = max(int(c["d_src"].max()) for c in cores_raw)

    # ---- class sizes m[w][d]: max over (core, partition) ----
    m = np.zeros((NW, dmax + 1), np.int64)
    for c in cores_raw:
        cnt = np.zeros((P, NW, dmax + 1), np.int64)
        np.add.at(cnt, (c["usrc_p"], c["win_of_usrc"], c["d_src"]), 1)
        m = np.maximum(m, cnt.max(axis=0))
    m[:, 0] = 0

    # ---- class layout: per window, d DESC ----
    x0_off = np.zeros((NW, dmax + 1), np.int64)
    x0_woff = np.zeros(NW + 1, np.int64)
    o0 = 0
    for w in range(NW):
        x0_woff[w] = o0
        for d in range(dmax, 0, -1):
            if m[w][d] == 0:
                continue
            x0_off[w][d] = o0
            o0 += int(m[w][d])
    x0_woff[NW] = o0
    CLS = int(o0)
    assert even(CLS) <= MAXELEMS, f"CLS={CLS}"

    # ---- expanded X layout: per window, sections r=0..dmax-1 ----
    # section r holds the r-th out-edge copy of every class with d > r
    # (a prefix of the window's d-desc class list).
    n_wr = np.zeros((NW, dmax), np.int64)
    for w in range(NW):
        for r in range(dmax):
            n_wr[w][r] = int(m[w][r + 1:].sum())
    xsec_rel = np.zeros((NW, dmax), np.int64)
    W_w = np.zeros(NW, np.int64)
    copy_list = []          # (w, dst_rel, n)
    for w in range(NW):
        o = 0
        for r in range(dmax):
            if n_wr[w][r] == 0:
                continue
            xsec_rel[w][r] = o
            copy_list.append((w, int(o), int(n_wr[w][r])))
            o += int(n_wr[w][r])
        W_w[w] = even(o)
    xw_off = np.r_[0, np.cumsum(W_w)]
    XW = int(xw_off[-1])

    # ---- layer maxima per (window, range) ----
    rw = cdiv(csh, NRANGE)
    widths = np.array([min(rw, csh - r * rw) for r in range(NRANGE)])
    Lmax = np.zeros((NW, NRANGE), np.int64)
    for c in cores_raw:
        r_v = c["c_v"] // rw
        np.maximum.at(Lmax, (c["w_e"], r_v), c["l_e"] + 1)
    Lmax = np.maximum(Lmax, 1)
    # enforce non-increasing in r (suffix max) for the prefix property
    for w in range(NW):
        for r in range(NRANGE - 2, -1, -1):
            Lmax[w][r] = max(Lmax[w][r], Lmax[w][r + 1])

    # ---- layer-major S layout per window ----
    secL_rel = np.zeros((NW, int(Lmax.max())), np.int64)
    W_l = np.zeros((NW, int(Lmax.max())), np.int64)
    S_w = np.zeros(NW, np.int64)
    for w in range(NW):
        o = 0
        for l in range(int(Lmax[w].max())):
            n_l = int((Lmax[w] > l).sum())
            wl = int(widths[:n_l].sum())
            secL_rel[w][l] = o
            W_l[w][l] = wl
            o += wl
        S_w[w] = even(o)
        assert S_w[w] <= MAXELEMS, f"S window {w} = {S_w[w]} > {MAXELEMS}"
    sm_off = np.r_[0, np.cumsum(S_w)]

    meta = dict(
        nc=nc, csh=csh, sh=sh, npad=npad, cf=cf, dmax=dmax, bpw=bpw,
        CLS=even(CLS), XW=XW, m=m, x0_off=x0_off, x0_woff=x0_woff,
        n_wr=n_wr, xsec_rel=xsec_rel, W_w=W_w, xw_off=xw_off,
        copy_list=copy_list, rw=rw, widths=widths, Lmax=Lmax,
        secL_rel=secL_rel, W_l=W_l, S_w=S_w, sm_off=sm_off,
    )
    per_core = [emit_core_arrays(c, meta) for c in cores_raw]
    return meta, per_core


def emit_core_arrays(c, meta):
    cf = meta["cf"]
    dmax = meta["dmax"]
    x0_off, x0_woff = meta["x0_off"], meta["x0_woff"]
    xsec_rel, W_w = meta["xsec_rel"], meta["W_w"]
    secL_rel = meta["secL_rel"]
    rw = meta["rw"]
    bpw = meta["bpw"]

    # class rank of each source within (p, w, d)
    ck = (c["usrc_p"] * NW + c["win_of_usrc"]) * (dmax + 1) + c["d_src"]
    cls_rank = _cumcount(ck)

    # ls1: gather state -> class layout
    ls1 = np.full((P, cf), -1, np.int16)
    tgt = x0_off[c["win_of_usrc"], c["d_src"]] + cls_rank
    assert tgt.max() < meta["CLS"]
    ls1[c["usrc_p"], c["usrc_q"]] = tgt.astype(np.int16)

    # window-relative class index per source, then per edge
    clsrel_src = (x0_off[c["win_of_usrc"], c["d_src"]]
                  - x0_woff[c["win_of_usrc"]] + cls_rank)
    si = c["src_inv"]
    xpos = xsec_rel[c["w_e"], c["r_in_src"]] + clsrel_src[si]

    # ls2: expanded X window -> transpose blocks
    ls2 = []
    for w in range(NW):
        a2 = np.full((P, int(W_w[w])), -1, np.int16)
        sel = c["w_e"] == w
        t2 = c["b_e"][sel] * P + c["j_e"][sel]
        assert len(t2) == 0 or t2.max() < bpw * P
        a2[c["p_s"][sel], xpos[sel]] = t2.astype(np.int16)
        ls2.append(a2)

    # ls3: transposed blocks -> layer-major S window
    r_v = c["c_v"] // rw
    ls3 = []
    for w in range(NW):
        arr = np.full((P, bpw * P), -1, np.int16)
        sel = c["w_e"] == w
        ipos = c["b_e"][sel] * P + c["p_s"][sel]
        t3 = secL_rel[w, c["l_e"][sel]] + c["c_v"][sel]
        assert len(t3) == 0 or t3.max() < meta["S_w"][w]
        arr[c["j_e"][sel], ipos] = t3.astype(np.int16)
        ls3.append(arr)

    return dict(ls1=ls1, ls2=ls2, ls3=ls3)


# ──────────────────────────────────────────────────────────────────────
# numpy emulation of one round (validation)
# ──────────────────────────────────────────────────────────────────────

def _emu_ls(data, idx, num_elems):
    Pp, n = idx.shape
    assert data.shape[0] == Pp and data.shape[1] >= n
    out = np.zeros((Pp, num_elems), data.dtype)
    for p in range(Pp):
        ii = idx[p].astype(np.int64)
        valid = ii >= 0
        assert len(np.unique(ii[valid])) == valid.sum(), "dup idx"
        out[p, ii[valid]] = data[p, :n][valid]
    return out


def emulate_round(w_full, meta, arrs, fdtype=np.float32):
    """w_full: [npad] pre-scaled state (device order). Returns u [SEGS,csh]
    = unscaled scatter-add for this core."""
    cf, csh = meta["cf"], meta["csh"]
    CLS, XW = meta["CLS"], meta["XW"]
    bpw = meta["bpw"]
    W_w, xw_off = meta["W_w"], meta["xw_off"]
    S_w, sm_off = meta["S_w"], meta["sm_off"]
    secL_rel, W_l, Lmax = meta["secL_rel"], meta["W_l"], meta["Lmax"]

    state = w_full.reshape(P, cf).astype(fdtype)
    x0 = _emu_ls(state, arrs["ls1"], CLS)

    X = np.zeros((P, XW), fdtype)
    for (w, dst, n) in meta["copy_list"]:
        X[:, xw_off[w] + dst: xw_off[w] + dst + n] = \
            x0[:, meta["x0_woff"][w]: meta["x0_woff"][w] + n]

    F = NW * bpw * P
    x2 = np.zeros((P, F), fdtype)
    for w in range(NW):
        o = _emu_ls(X[:, xw_off[w]:xw_off[w] + W_w[w]], arrs["ls2"][w],
                    bpw * P)
        x2[:, w * bpw * P:(w + 1) * bpw * P] = o

    xt = np.zeros((P, F), fdtype)
    for b in range(NW * bpw):
        xt[:, b * P:(b + 1) * P] = x2[:, b * P:(b + 1) * P].T

    y = np.zeros((NW, P, csh), fdtype)
    for w in range(NW):
        sm = _emu_ls(xt[:, w * bpw * P:(w + 1) * bpw * P], arrs["ls3"][w],
                     int(S_w[w]))
        for l in range(int(Lmax[w].max())):
            wl = int(W_l[w][l])
            y[w][:, :wl] += sm[:, int(secL_rel[w][l]):int(secL_rel[w][l]) + wl]

    u = np.zeros((SEGS, csh), fdtype)
    for w in range(NW):
        u += y[w].reshape(SEGS, R4, csh).sum(axis=1)
    return u


# ──────────────────────────────────────────────────────────────────────
# preprocess / inputs
# ──────────────────────────────────────────────────────────────────────

def next_pow2(x):
    p = 1
    while p < x:
        p *= 2
    return p


def preprocess(x, edge_index, batch, nc_count=8, n_graphs=128):
    n_nodes = x.shape[0]
    row = np.asarray(edge_index[0], np.int64)
    col = np.asarray(edge_index[1], np.int64)
    batch = np.asarray(batch, np.int64)

    csh, sh, npad, cf = build_layout(n_nodes, nc_count)
    deg = np.bincount(col, minlength=npad).astype(np.int64)
    flat, layout = relabel(deg, n_nodes, nc_count)
    re, ve = flat[row], flat[col]

    meta = per_core = None
    for bpw in (6, 7, 8):
        try:
            cores_raw = []
            for c in range(nc_count):
                mm = (ve // sh) == c
                cores_raw.append(build_core(c, re[mm], ve[mm], layout, bpw))
            meta, per_core = finalize_cores(cores_raw, layout, bpw)
            break
        except (AssertionError, RuntimeError):
            continue
    assert meta is not None, "routing build failed for all bpw"

    # device-order node arrays
    inv = np.empty(npad, np.int64)
    inv[flat] = np.arange(npad)
    deg_dev = deg[inv].astype(np.float32)
    batch_dev = np.full(npad, -1, np.int64)
    batch_dev[flat[:n_nodes]] = batch[:n_nodes]

    # ---- pooling structures ----
    g0 = np.zeros(nc_count, np.int64)
    ngl = np.zeros(nc_count, np.int64)
    wg_max = 0
    for c in range(nc_count):
        bd = batch_dev[c * sh:(c + 1) * sh]
        real = bd >= 0
        gmin, gmax = (int(bd[real].min()), int(bd[real].max())) \
            if real.any() else (0, 0)
        g0[c], ngl[c] = gmin, gmax - gmin + 1
        fin = np.arange(sh)
        s = fin // csh
        cnt = np.zeros((SEGS, int(ngl[c])), np.int64)
        np.add.at(cnt, (s[real], bd[real] - gmin), 1)
        wg_max = max(wg_max, int(cnt.max()))
    NGLP = int(ngl.max())
    WGP = next_pow2(wg_max)
    GPH = max(1, min(MAXELEMS // WGP, NGLP))
    NPH = cdiv(NGLP, GPH)
    PHALF = GPH * WGP
    assert NPH * GPH <= 32, f"pool pad {NPH * GPH} > 32"

    pool_idx = []
    gms = [np.zeros((P, P), np.float32) for _ in range(2)]
    for c in range(nc_count):
        bd = batch_dev[c * sh:(c + 1) * sh]
        fin = np.arange(sh)
        s, cc = fin // csh, fin % csh
        lg = bd - g0[c]
        arrs = []
        rank = np.zeros(sh, np.int64)
        real = bd >= 0
        key = s * 4096 + lg
        rank[real] = _cumcount(key[real])
        assert rank.max() < WGP
        for h in range(NPH):
            a = np.full((SEGS, csh), -1, np.int16)
            selh = real & (lg >= h * GPH) & (lg < (h + 1) * GPH)
            tgt = (lg[selh] - h * GPH) * WGP + rank[selh]
            assert len(tgt) == 0 or tgt.max() < PHALF
            a[s[selh], cc[selh]] = tgt.astype(np.int16)
            arrs.append(a)
        pool_idx.append(arrs)
        for li in range(NPH * GPH):
            g = g0[c] + li
            if li < int(ngl[c]) and g < n_graphs:
                fp = c * 32 + li
                gms[fp % 2][fp // 2, g] = 1.0

    pool_meta = dict(NGLP=NPH * GPH, NGH=GPH, WGP=WGP, NPH=NPH, PHALF=PHALF,
                     g0=g0)
    return meta, per_core, pool_meta, pool_idx, gms, flat, deg_dev, layout


def _degq(degsh):
    sh = len(degsh)
    nblk = cdiv(sh, P)
    pad = np.zeros(nblk * P, np.float32)
    pad[:sh] = degsh
    return np.ascontiguousarray(pad.reshape(nblk, P).T)


def make_inputs(meta, pool_meta, per_core, pool_idx, gms, flat, deg_dev,
                x, weights, n_max, n_graphs=128):
    csh, sh, npad, cf = meta["csh"], meta["sh"], meta["npad"], meta["cf"]
    nc_count = meta["nc"]
    n_nodes = x.shape[0]

    xdev = np.zeros((npad, x.shape[1]), np.float32)
    xdev[flat[:n_nodes]] = x
    in_maps = []
    for c in range(nc_count):
        im = dict(
            xT=np.ascontiguousarray(xdev[c * sh:(c + 1) * sh].T).astype(BF16),
            degs=deg_dev[c * sh:(c + 1) * sh].reshape(SEGS, csh),
            degq=_degq(deg_dev[c * sh:(c + 1) * sh]),
            ls1=per_core[c]["ls1"],
            w1t=np.ascontiguousarray(weights["W1"].T),
            w2t=np.ascontiguousarray(weights["W2"].T),
            w3t=np.ascontiguousarray(weights["W3"].T),
            w4t=np.ascontiguousarray(weights["W4"].T),
            wl=np.ascontiguousarray(weights["Wl"]).reshape(64, 1),
            bl=np.asarray(weights["bl"], np.float32).reshape(1, 1),
            invn=np.asarray([[1.0 / np.float32(n_max)]], np.float32),
            selb=np.repeat(np.eye(SEGS, dtype=np.float32),
                           R4, axis=0).astype(BF16),
            identb=np.eye(P, dtype=np.float32).astype(BF16),
            ones16=np.ones((SEGS, 1), np.float32),
            gm0=gms[0], gm1=gms[1],
        )
        for k in range(1, 5):
            im[f"b{k}"] = np.asarray(weights[f"b{k}"], np.float32).reshape(64, 1)
        for w in range(NW):
            im[f"ls2_{w}"] = per_core[c]["ls2"][w]
            im[f"ls3_{w}"] = per_core[c]["ls3"][w]
        for h in range(pool_meta["NPH"]):
            im[f"pool_{h}"] = pool_idx[c][h]
        in_maps.append(im)
    return in_maps


def reference_numpy(x, edge_index, batch, weights, n_graphs=128):
    row = np.asarray(edge_index[0]); col = np.asarray(edge_index[1])
    N = x.shape[0]
    deg = np.bincount(col, minlength=N).astype(np.float64)
    dinv = np.where(deg > 0, deg ** -0.5, 0.0)
    norm = dinv[row] * dinv[col]
    h = x.astype(np.float64)
    for k in range(1, 5):
        W = weights[f"W{k}"]
        b = weights[f"b{k}"]
        hw = h @ W
        msg = norm[:, None] * hw[row]
        out = np.zeros((N, hw.shape[1]))
        np.add.at(out, col, msg)
        h = out + b
    sums = np.zeros((n_graphs, h.shape[1]))
    np.add.at(sums, batch, h)
    counts = np.bincount(batch, minlength=n_graphs)
    pooled = sums / counts.max()
    return (pooled @ weights["Wl"] + weights["bl"]).astype(np.float32)


# ──────────────────────────────────────────────────────────────────────
# device kernel
# ──────────────────────────────────────────────────────────────────────
from contextlib import ExitStack

import concourse.bass as bass
import concourse.tile as tile
from concourse import bacc, mybir

FP32 = mybir.dt.float32
BF16D = mybir.dt.bfloat16
I16 = mybir.dt.int16
AT = mybir.ActivationFunctionType
OP = mybir.AluOpType


def build_kernel(meta, pool_meta, n_graphs=128):
    csh, sh, npad, cf = meta["csh"], meta["sh"], meta["npad"], meta["cf"]
    CLS, XW = meta["CLS"], meta["XW"]
    bpw = meta["bpw"]
    W_w, xw_off = meta["W_w"], meta["xw_off"]
    S_w, sm_off = meta["S_w"], meta["sm_off"]
    secL_rel, W_l, Lmax = meta["secL_rel"], meta["W_l"], meta["Lmax"]
    NB = NW * bpw                       # total transpose blocks
    NGLP, WGP = pool_meta["NGLP"], pool_meta["WGP"]
    PHALF = pool_meta["PHALF"]
    NPH = pool_meta["NPH"]
    core_ids = list(range(meta["nc"]))
    cuts = [(0, min(512, csh))]
    if csh > 512:
        cuts.append((512, csh))

    nc = bacc.Bacc("TRN2", target_bir_lowering=False, debug=False,
                   num_devices=meta["nc"])

    def din(name, shape, dt=FP32):
        return nc.declare_dram_parameter(name, list(shape), dt, isOutput=False)

    # ---- inputs ----
    xT_in = din("xT", [P, sh], BF16D)
    NBLK = cdiv(sh, P)                  # 128-node blocks for the s0 matvec
    lastw = sh - (NBLK - 1) * P
    degs_in = din("degs", [SEGS, csh])
    degq_in = din("degq", [P, NBLK])
    ls1_in = din("ls1", [P, cf], I16)
    ls2_in = [din(f"ls2_{w}", [P, int(W_w[w])], I16) for w in range(NW)]
    ls3_in = [din(f"ls3_{w}", [P, bpw * P], I16) for w in range(NW)]
    pool_in = [din(f"pool_{h}", [SEGS, csh], I16) for h in range(NPH)]
    w1t_in = din("w1t", [64, 128])
    w2t_in = din("w2t", [64, 64])
    w3t_in = din("w3t", [64, 64])
    w4t_in = din("w4t", [64, 64])
    wl_in = din("wl", [64, 1])
    b_in = [din(f"b{k}", [64, 1]) for k in range(1, 5)]
    bl_in = din("bl", [1, 1])
    invn_in = din("invn", [1, 1])
    selb_in = din("selb", [P, SEGS], BF16D)
    identb_in = din("identb", [P, P], BF16D)
    ones16_in = din("ones16", [SEGS, 1])
    gm_in = [din(f"gm{i}", [P, P]) for i in range(2)]
    out_ext = nc.declare_dram_parameter("out", [n_graphs], FP32, isOutput=True)

    # ---- internal DRAM ----
    sh_dram = nc.dram_tensor("sh_dram", [sh], BF16D)
    full_dram = nc.dram_tensor("full_dram", [npad], BF16D, addr_space="Shared")
    part_dram = nc.dram_tensor("part_dram", [32], FP32)
    warm_in = nc.dram_tensor("warm_in", [32], FP32)
    warm_out = nc.dram_tensor("warm_out", [256], FP32, addr_space="Shared")
    partall_dram = nc.dram_tensor("partall_dram", [256], FP32,
                                  addr_space="Shared")

    with tile.TileContext(nc) as tc:
        with ExitStack() as ctx:
            pool = ctx.enter_context(tc.tile_pool(name="p", bufs=1))
            tp = ctx.enter_context(tc.tile_pool(name="tp", bufs=2,
                                                space="PSUM"))
            up = ctx.enter_context(tc.tile_pool(name="up", bufs=2,
                                                space="PSUM"))

            # persistent tiles
            state = pool.tile([P, cf], BF16D)
            x0 = pool.tile([P, CLS], BF16D)
            X = pool.tile([P, XW], BF16D)
            x2 = pool.tile([P, NB * P], BF16D)
            xt = pool.tile([P, NB * P], BF16D)
            sm = pool.tile([P, int(sm_off[-1])], BF16D)
            yw = [pool.tile([P, csh], BF16D, name=f"y{w}") for w in range(NW)]
            dinvs = pool.tile([SEGS, csh], FP32)
            dinvs2 = pool.tile([SEGS, csh], FP32)
            bd = [pool.tile([SEGS, csh], FP32, name=f"bd{k}")
                  for k in range(NW)]
            ts1 = pool.tile([SEGS, csh], FP32)
            wout = pool.tile([SEGS, csh], BF16D)
            s4 = pool.tile([SEGS, csh], FP32)
            degs = pool.tile([SEGS, csh], FP32)
            tmp16 = pool.tile([SEGS, csh], FP32)
            degq = pool.tile([P, NBLK], FP32)
            dinvq = pool.tile([P, NBLK], FP32)
            tmpq = pool.tile([P, NBLK], FP32)
            wout0 = pool.tile([P, NBLK], BF16D)
            selb = pool.tile([P, SEGS], BF16D)
            identb = pool.tile([P, P], BF16D)
            ones16 = pool.tile([SEGS, 1], FP32)
            gm = [pool.tile([P, P], FP32, name=f"gm{i}") for i in range(2)]
            ls1 = pool.tile([P, cf], I16)
            ls2 = [pool.tile([P, int(W_w[w])], I16, name=f"ls2t{w}")
                   for w in range(NW)]
            ls3 = [pool.tile([P, bpw * P], I16, name=f"ls3t{w}")
                   for w in range(NW)]
            plidx = [pool.tile([SEGS, csh], I16, name=f"plidx{h}")
                     for h in range(NPH)]
            poolsrc = pool.tile([SEGS, csh], BF16D)
            poolbuf = pool.tile([SEGS, NPH * PHALF], BF16D)
            poolf32 = pool.tile([SEGS, NPH * PHALF], FP32)
            p16 = pool.tile([SEGS, NGLP], FP32)
            part_sb = pool.tile([1, 32], FP32)
            partall = pool.tile([P, 2], FP32)
            outrow = pool.tile([1, n_graphs], FP32)
            wts = {
                "w1t": pool.tile([64, 128], FP32, name="w1t_t"),
                "w2t": pool.tile([64, 64], FP32, name="w2t_t"),
                "w3t": pool.tile([64, 64], FP32, name="w3t_t"),
                "w4t": pool.tile([64, 64], FP32, name="w4t_t"),
                "wl": pool.tile([64, 1], FP32, name="wl_t"),
            }
            bs = [pool.tile([64, 1], FP32, name=f"bs{k}") for k in range(4)]
            blt = pool.tile([1, 1], FP32)
            invn = pool.tile([1, 1], FP32)
            cvec = {
                "c3": pool.tile([64, 1], FP32, name="c3t"),
                "c2": pool.tile([64, 1], FP32, name="c2t"),
                "c1": pool.tile([64, 1], FP32, name="c1t"),
                "c0": pool.tile([128, 1], FP32, name="c0t"),
            }
            c0b = pool.tile([128, 1], BF16D)
            betas = pool.tile([1, 4], FP32)
            ones116 = pool.tile([1, 16], FP32)
            betas16 = pool.tile([SEGS, 4], FP32)

            # ---- loads ----
            warmsb = pool.tile([1, 32], FP32)
            nc.vector.memset(warmsb[:], 0.0)
            nc.sync.dma_start(warm_in[:].rearrange("(a b) -> a b", a=1),
                              warmsb[:])
            nc.gpsimd.collective_compute(
                "AllGather", OP.bypass, replica_groups=[core_ids],
                ins=[warm_in[:]], outs=[warm_out[:]],
            )
            nc.sync.dma_start(ls1[:], ls1_in[:])
            for w in range(NW):
                nc.sync.dma_start(ls2[w][:], ls2_in[w][:])
                nc.sync.dma_start(ls3[w][:], ls3_in[w][:])
            for h in range(NPH):
                nc.sync.dma_start(plidx[h][:], pool_in[h][:])
            nc.sync.dma_start(selb[:], selb_in[:])
            nc.sync.dma_start(identb[:], identb_in[:])
            nc.sync.dma_start(ones16[:], ones16_in[:])
            for i in range(2):
                nc.sync.dma_start(gm[i][:], gm_in[i][:])
            for k, t in wts.items():
                nc.sync.dma_start(t[:], {"w1t": w1t_in, "w2t": w2t_in,
                                         "w3t": w3t_in, "w4t": w4t_in,
                                         "wl": wl_in}[k][:])
            for k in range(4):
                nc.sync.dma_start(bs[k][:], b_in[k][:])
            nc.sync.dma_start(blt[:], bl_in[:])
            nc.sync.dma_start(invn[:], invn_in[:])
            nc.sync.dma_start(degs[:], degs_in[:])
            nc.sync.dma_start(degq[:], degq_in[:])

            # ---- dinv = rsqrt(deg + (deg==0)) * (deg>0) ----
            def make_dinv(dst, deg_t, tmp):
                nc.vector.tensor_scalar(tmp[:], deg_t[:], 0.0, None,
                                        OP.is_equal)
                nc.vector.tensor_tensor(tmp[:], tmp[:], deg_t[:], OP.add)
                nc.scalar.activation(tmp[:], tmp[:], AT.Sqrt)
                nc.vector.reciprocal(tmp[:], tmp[:])
                nc.vector.tensor_scalar(dst[:], deg_t[:], 0.0, None, OP.is_gt)
                nc.vector.tensor_tensor(dst[:], dst[:], tmp[:], OP.mult)

            make_dinv(dinvs, degs, tmp16)
            make_dinv(dinvq, degq, tmpq)
            nc.vector.tensor_tensor(dinvs2[:], dinvs[:], dinvs[:], OP.mult)

            # ---- c chain + betas ----
            pc = tp.tile([128, 4], FP32, tag="ops", bufs=1)
            nc.tensor.matmul(pc[0:64, 0:1], wts["w4t"][:], wts["wl"][:],
                             start=True, stop=True)
            nc.vector.tensor_copy(cvec["c3"][:], pc[0:64, 0:1])
            nc.tensor.matmul(pc[0:64, 1:2], wts["w3t"][:], cvec["c3"][:],
                             start=True, stop=True)
            nc.vector.tensor_copy(cvec["c2"][:], pc[0:64, 1:2])
            nc.tensor.matmul(pc[0:64, 2:3], wts["w2t"][:], cvec["c2"][:],
                             start=True, stop=True)
            nc.vector.tensor_copy(cvec["c1"][:], pc[0:64, 2:3])
            nc.tensor.matmul(pc[0:128, 3:4], wts["w1t"][:], cvec["c1"][:],
                             start=True, stop=True)
            nc.vector.tensor_copy(cvec["c0"][:], pc[0:128, 3:4])
            nc.vector.tensor_copy(c0b[:], cvec["c0"][:])
            pb = tp.tile([1, 4], FP32, tag="ops", bufs=1)
            for k, cn in enumerate(["c1", "c2", "c3"]):
                nc.tensor.matmul(pb[0:1, k:k + 1], bs[k][:], cvec[cn][:],
                                 start=True, stop=True)
            nc.tensor.matmul(pb[0:1, 3:4], bs[3][:], wts["wl"][:],
                             start=True, stop=True)
            nc.vector.tensor_copy(betas[:], pb[:])
            nc.vector.memset(ones116[:], 1.0)
            pbb = tp.tile([SEGS, 4], FP32, tag="ops", bufs=1)
            nc.tensor.matmul(pbb[:], ones116[:], betas[:], start=True,
                             stop=True)
            nc.vector.tensor_copy(betas16[:], pbb[:])
            # bd[k] = dinvs * beta_k   (k = 0..2 for rounds 0..2)
            for k in range(NW):
                nc.vector.tensor_scalar(bd[k][:], dinvs[:],
                                        betas16[:, k:k + 1], None, OP.mult)

            # ---- s0 = x @ c0: stationary 128-node blocks, col k per block ----
            pu0 = up.tile([P, NBLK], FP32, tag="pu0", bufs=1, name="pu_s0")
            BPC = 25                     # blocks per xq chunk
            for q in range(cdiv(NBLK, BPC)):
                b0_, b1_ = q * BPC, min((q + 1) * BPC, NBLK)
                wq = min(b1_ * P, sh) - b0_ * P
                xq = pool.tile([P, BPC * P], BF16D, tag="xq", bufs=2,
                               name=f"xq{q}")
                nc.sync.dma_start(xq[:, 0:wq],
                                  xT_in[:, b0_ * P:b0_ * P + wq])
                for b in range(b0_, b1_):
                    bw = min(P, sh - b * P)
                    nc.tensor.matmul(
                        pu0[0:bw, b:b + 1],
                        xq[:, (b - b0_) * P:(b - b0_) * P + bw],
                        c0b[:], start=True, stop=True)
            nc.vector.tensor_tensor(wout0[:], pu0[:], dinvq[:], OP.mult)

            # ---- rounds ----
            for rnd in range(4):
                if rnd == 0:
                    nc.sync.dma_start(
                        sh_dram[0:(NBLK - 1) * P].rearrange(
                            "(k m) -> m k", m=P), wout0[:, 0:NBLK - 1])
                    nc.sync.dma_start(
                        sh_dram[(NBLK - 1) * P:sh].rearrange(
                            "(k m) -> m k", m=lastw),
                        wout0[0:lastw, NBLK - 1:NBLK])
                else:
                    nc.sync.dma_start(
                        sh_dram[:].rearrange("(a b) -> a b", a=SEGS), wout[:])
                nc.gpsimd.collective_compute(
                    "AllGather", OP.bypass, replica_groups=[core_ids],
                    ins=[sh_dram[:]], outs=[full_dram[:]],
                )
                nc.sync.dma_start(
                    state[:], full_dram[:].rearrange("(p c) -> p c", p=P))

                nc.gpsimd.local_scatter(
                    x0[:], state[:], ls1[:],
                    channels=P, num_elems=CLS, num_idxs=cf)
                for (w, dst, n) in meta["copy_list"]:
                    nc.vector.tensor_copy(
                        X[:, int(xw_off[w]) + dst:int(xw_off[w]) + dst + n],
                        x0[:, int(meta["x0_woff"][w]):
                           int(meta["x0_woff"][w]) + n])

                def emit_ls2(w):
                    nc.gpsimd.local_scatter(
                        x2[:, w * bpw * P:(w + 1) * bpw * P],
                        X[:, int(xw_off[w]):int(xw_off[w]) + int(W_w[w])],
                        ls2[w][:], channels=P, num_elems=bpw * P,
                        num_idxs=int(W_w[w]))

                def emit_transp(w):
                    blocks = list(range(w * bpw, (w + 1) * bpw))
                    for g0_ in range(0, len(blocks), 4):
                        grp = blocks[g0_:g0_ + 4]
                        pt = tp.tile([P, 4 * P], BF16D, tag="ptr",
                                     name=f"pt{rnd}_{w}_{g0_}")
                        for k, b in enumerate(grp):
                            nc.tensor.transpose(pt[:, k * P:(k + 1) * P],
                                                x2[:, b * P:(b + 1) * P],
                                                identb[:])
                        nc.vector.tensor_copy(
                            xt[:, grp[0] * P:(grp[-1] + 1) * P],
                            pt[:, 0:len(grp) * P])

                def emit_ls3(w):
                    nc.gpsimd.local_scatter(
                        sm[:, int(sm_off[w]):int(sm_off[w]) + int(S_w[w])],
                        xt[:, w * bpw * P:(w + 1) * bpw * P],
                        ls3[w][:], channels=P, num_elems=int(S_w[w]),
                        num_idxs=bpw * P)

                # gpsimd order: ls2_0, ls2_1, ls3_0, ls2_2, ls3_1, ls3_2
                emit_ls2(0)
                emit_ls2(1)
                emit_transp(0)
                emit_ls3(0)
                emit_ls2(2)
                emit_transp(1)
                emit_ls3(1)
                emit_transp(2)
                emit_ls3(2)

                # layer sums into y_w (bf16)
                for w in range(NW):
                    base = int(sm_off[w])
                    nc.vector.tensor_copy(
                        yw[w][:, 0:int(W_l[w][0])],
                        sm[:, base:base + int(W_l[w][0])])
                    for l in range(1, int(Lmax[w].max())):
                        wl_ = int(W_l[w][l])
                        o = base + int(secL_rel[w][l])
                        nc.vector.tensor_tensor(
                            yw[w][:, 0:wl_], yw[w][:, 0:wl_],
                            sm[:, o:o + wl_], OP.add)

                # segment reduction: psum-accumulated sel matmuls
                pu = up.tile([SEGS, csh], FP32, tag="pu", name=f"pu{rnd}")
                for (a, b2) in cuts:
                    for w in range(NW):
                        nc.tensor.matmul(pu[:, a:b2], selb[:],
                                         yw[w][:, a:b2],
                                         start=(w == 0), stop=(w == NW - 1))

                if rnd < 3:
                    nc.vector.tensor_tensor(ts1[:], pu[:], dinvs2[:], OP.mult)
                    nc.vector.tensor_tensor(wout[:], ts1[:], bd[rnd][:],
                                            OP.add)
                else:
                    nc.vector.tensor_tensor(ts1[:], pu[:], dinvs[:], OP.mult)
                    nc.vector.tensor_scalar(s4[:], ts1[:],
                                            betas16[:, 3:4], None, OP.add)

            # ---- pooling ----
            nc.vector.tensor_copy(poolsrc[:], s4[:])
            for h in range(NPH):
                nc.gpsimd.local_scatter(
                    poolbuf[:, h * PHALF:(h + 1) * PHALF],
                    poolsrc[:], plidx[h][:],
                    channels=SEGS, num_elems=PHALF, num_idxs=csh)
            nc.vector.tensor_copy(poolf32[:], poolbuf[:])
            wgp = WGP
            a = poolf32[:].rearrange("s (g t) -> s g t", t=WGP)
            while wgp > 1:
                hw = wgp // 2
                nc.vector.tensor_tensor(
                    a[:, :, 0:hw], a[:, :, 0:hw], a[:, :, hw:wgp], OP.add)
                wgp = hw
            nc.vector.tensor_copy(
                p16[:], a[:, :, 0:1].rearrange("s g t -> s (g t)"))
            pp = tp.tile([1, NGLP], FP32, tag="ops", bufs=1)
            nc.tensor.matmul(pp[:], ones16[:], p16[:], start=True, stop=True)
            nc.vector.memset(part_sb[:], 0.0)
            nc.vector.tensor_copy(part_sb[:, 0:NGLP], pp[:])
            nc.sync.dma_start(part_dram[:].rearrange("(a b) -> a b", a=1),
                              part_sb[:])
            nc.gpsimd.collective_compute(
                "AllGather", OP.bypass, replica_groups=[core_ids],
                ins=[part_dram[:]], outs=[partall_dram[:]],
            )
            nc.sync.dma_start(partall[:],
                              partall_dram[:].rearrange("(p c) -> p c", p=P))
            po_ = tp.tile([1, n_graphs], FP32, tag="ops", bufs=1)
            nc.tensor.matmul(po_[:], partall[:, 0:1], gm[0][:],
                             start=True, stop=False)
            nc.tensor.matmul(po_[:], partall[:, 1:2], gm[1][:],
                             start=False, stop=True)
            nc.vector.tensor_copy(outrow[:], po_[:])
            nc.vector.tensor_scalar(outrow[:], outrow[:], invn[0:1, 0:1],
                                    None, OP.mult)
            nc.vector.tensor_scalar(outrow[:], outrow[:], blt[0:1, 0:1],
                                    None, OP.add)
            nc.sync.dma_start(out_ext[:].rearrange("(a b) -> a b", a=1),
                              outrow[:])
    return nc


# ─── entry point ───

def kernel(x, edge_index, batch, W1, b1, W2, b2, W3, b3, W4, b4, Wl, bl):
    from concourse.bass_utils import run_bass_kernel_spmd

    x = np.asarray(x, np.float32)
    edge_index = np.asarray(edge_index)
    batch = np.asarray(batch)
    weights = dict(W1=np.asarray(W1, np.float32), W2=np.asarray(W2, np.float32),
                   W3=np.asarray(W3, np.float32), W4=np.asarray(W4, np.float32),
                   Wl=np.asarray(Wl, np.float32),
                   b1=np.asarray(b1, np.float32), b2=np.asarray(b2, np.float32),
                   b3=np.asarray(b3, np.float32), b4=np.asarray(b4, np.float32),
                   bl=np.asarray(bl, np.float32))
    n_graphs = 128

    meta, per_core, pool_meta, pool_idx, gms, flat, deg_dev, layout = \
        preprocess(x, edge_index, batch, 8, n_graphs)
    n_max = int(np.bincount(np.asarray(batch, np.int64),
                            minlength=n_graphs).max())
    in_maps = make_inputs(meta, pool_meta, per_core, pool_idx, gms, flat,
                          deg_dev, x, weights, n_max, n_graphs)
    nc = build_kernel(meta, pool_meta, n_graphs)
    nc.finalize()
    res = run_bass_kernel_spmd(nc, in_maps, core_ids=list(range(8)),
                               trace=False)
    return res.results[0]["out"].reshape(n_graphs, 1).astype(np.float32)


# revision 13
# speedup vs baseline: 1.6365x; 1.1321x over previous
"""Trainium2 kernel for nn_GCNRegression: linear-GCN scalar collapse, bf16.

The model is linear (no activation), so 4 GCN layers + mean-pool + linear
head collapse exactly to scalar propagation through the graph:
    c0 = W1 @ W2 @ W3 @ W4 @ Wl;  s0 = x @ c0
    s_k = dinv * (Adj^T @ (dinv * s_{k-1})) + b_k . c_k
    out[g] = sum_{v in g} s4[v] / n_max + bl
8 NeuronCores, destination-sharded edges. Per round: AllGather the
dinv-pre-scaled bf16 state, per-partition local_scatter routing (gather
classes -> expand by out-degree -> scatter into transpose blocks), PE
transposes, local_scatter into a layer-major segment layout, DVE layer
sums, one PSUM-accumulated sel matmul, scale + bias.  All index arrays
are host-precomputed from the edge list; values move as bf16 (single
int16 slot per value in every local_scatter).
"""

import sys

sys.path.insert(0, "/opt/trn_rl_repo")

import numpy as np
import ml_dtypes

BF16 = ml_dtypes.bfloat16

P = 128          # partitions
SEGS = 16        # shard rows (psum partitions)
R4 = P // SEGS   # 8 partition rows per segment
NW = 3           # windows (= LS2/LS3 call count)
NRANGE = 16      # column ranges for layer caps
MAXELEMS = 2046  # local_scatter out-region limit (num_elems*32 < 2^16)


def cdiv(a, b):
    return (a + b - 1) // b


def even(x):
    return x + (x & 1)


def _cumcount(keys):
    """Rank of each element within its key group (stable, array order)."""
    order = np.argsort(keys, kind="stable")
    sk = keys[order]
    grp_start = np.r_[0, np.flatnonzero(sk[1:] != sk[:-1]) + 1]
    sizes = np.diff(np.r_[grp_start, len(keys)])
    cum = np.arange(len(keys)) - np.repeat(grp_start, sizes)
    out = np.empty(len(keys), np.int64)
    out[order] = cum
    return out


def build_layout(n_nodes, nc):
    csh = cdiv(n_nodes, nc * SEGS)
    sh = SEGS * csh
    npad = nc * sh
    cf = npad // P
    return csh, sh, npad, cf


def relabel(deg, n_nodes, nc):
    """Shard by original id; within shard sort by in-degree desc; lay
    column-major into [SEGS, CSH]. Returns flat[] over padded ids."""
    csh, sh, npad, cf = build_layout(n_nodes, nc)
    flat = np.empty(npad, np.int64)
    for c in range(nc):
        ids = np.arange(c * sh, (c + 1) * sh)
        order = np.argsort(-deg[ids], kind="stable")
        t = np.empty(len(ids), np.int64)
        t[order] = np.arange(len(ids))
        s, cc = t % SEGS, t // SEGS
        flat[ids] = c * sh + s * csh + cc
    return flat, (csh, sh, npad, cf)


def build_core(core, re, ve, layout, bpw):
    """Per-core routing. re/ve: device-flat src/dst positions."""
    csh, sh, npad, cf = layout
    E = len(re)
    p_s = re // cf
    q_s = re % cf
    fin = ve - core * sh
    s_v = fin // csh
    c_v = fin % csh

    # ---- window per source: per (partition, count-desc) round robin ----
    usrc, src_inv, src_cnt = np.unique(re, return_inverse=True,
                                       return_counts=True)
    usrc_p = usrc // cf
    so = np.lexsort((-src_cnt, usrc_p))
    rank_in_p = _cumcount(usrc_p[so])
    win_of_usrc = np.empty(len(usrc), np.int64)
    win_of_usrc[so] = rank_in_p % NW
    w_e = win_of_usrc[src_inv]

    # ---- initial j: balanced round-robin within (v, w) ----
    jr = _cumcount(fin * NW + w_e) % R4
    j_e = s_v * R4 + jr

    # ---- repair (p_s, w, j) column loads to <= bpw ----
    def psj(j):
        return (p_s * NW + w_e) * P + j

    cnt = np.bincount(psj(j_e), minlength=P * NW * P)
    vwj_key_all = (fin * NW + w_e) * P
    vwj = {}
    vk = vwj_key_all + j_e
    uk, uc = np.unique(vk, return_counts=True)
    vwj = dict(zip(uk.tolist(), uc.tolist()))
    for _try in range(400):
        rank = _cumcount(psj(j_e))
        move = np.flatnonzero(rank >= bpw)
        if len(move) == 0:
            break
        for i in move:
            base = s_v[i] * R4
            pw = (p_s[i] * NW + w_e[i]) * P
            best = None
            for r in range(R4):
                jv = base + r
                if jv == j_e[i]:
                    continue
                ld = cnt[pw + jv]
                if ld >= bpw:
                    continue
                nv = vwj.get(vwj_key_all[i] + jv, 0)
                key = (nv, ld)
                if best is None or key < best[0]:
                    best = (key, jv)
            if best is None:
                ld0 = [cnt[pw + base + r] for r in range(R4)]
                jv = base + int(np.argmin(ld0))
            else:
                jv = best[1]
            vwj[vwj_key_all[i] + j_e[i]] -= 1
            cnt[pw + j_e[i]] -= 1
            j_e[i] = jv
            cnt[pw + jv] += 1
            vwj[vwj_key_all[i] + jv] = vwj.get(vwj_key_all[i] + jv, 0) + 1
    else:
        raise RuntimeError("j balance failed")
    assert np.bincount(psj(j_e), minlength=P * NW * P).max() <= bpw

    b_e = _cumcount(psj(j_e))                       # block rank in [0,bpw)
    l_e = _cumcount((fin * NW + w_e) * P + j_e)     # layer rank per (v,w,j)
    r_in_src = _cumcount(src_inv)                   # edge rank within source

    return dict(
        E=E, p_s=p_s, q_s=q_s, s_v=s_v, c_v=c_v, w_e=w_e, j_e=j_e,
        b_e=b_e, l_e=l_e, r_in_src=r_in_src,
        usrc=usrc, usrc_p=usrc_p, usrc_q=usrc % cf, d_src=src_cnt,
        win_of_usrc=win_of_usrc, src_inv=src_inv,
    )


def finalize_cores(cores_raw, layout, bpw):
    csh, sh, npad, cf = layout
    nc = len(cores_raw)
    dmax = max(int(c["d_src"].max()) for c in cores_raw)

    # ---- class sizes m[w][d]: max over (core, partition) ----
    m = np.zeros((NW, dmax + 1), np.int64)
    for c in cores_raw:
        cnt = np.zeros((P, NW, dmax + 1), np.int64)
        np.add.at(cnt, (c["usrc_p"], c["win_of_usrc"], c["d_src"]), 1)
        m = np.maximum(m, cnt.max(axis=0))
    m[:, 0] = 0

    # ---- class layout: per window, d DESC ----
    x0_off = np.zeros((NW, dmax + 1), np.int64)
    x0_woff = np.zeros(NW + 1, np.int64)
    o0 = 0
    for w in range(NW):
        x0_woff[w] = o0
        for d in range(dmax, 0, -1):
            if m[w][d] == 0:
                continue
            x0_off[w][d] = o0
            o0 += int(m[w][d])
    x0_woff[NW] = o0
    CLS = int(o0)
    assert even(CLS) <= MAXELEMS, f"CLS={CLS}"

    # ---- expanded X layout: per window, sections r=0..dmax-1 ----
    # section r holds the r-th out-edge copy of every class with d > r
    # (a prefix of the window's d-desc class list).
    n_wr = np.zeros((NW, dmax), np.int64)
    for w in range(NW):
        for r in range(dmax):
            n_wr[w][r] = int(m[w][r + 1:].sum())
    xsec_rel = np.zeros((NW, dmax), np.int64)
    W_w = np.zeros(NW, np.int64)
    copy_list = []          # (w, dst_rel, n)
    for w in range(NW):
        o = 0
        for r in range(dmax):
            if n_wr[w][r] == 0:
                continue
            xsec_rel[w][r] = o
            copy_list.append((w, int(o), int(n_wr[w][r])))
            o += int(n_wr[w][r])
        W_w[w] = even(o)
    xw_off = np.r_[0, np.cumsum(W_w)]
    XW = int(xw_off[-1])

    # ---- layer maxima per (window, range) ----
    rw = cdiv(csh, NRANGE)
    widths = np.array([min(rw, csh - r * rw) for r in range(NRANGE)])
    Lmax = np.zeros((NW, NRANGE), np.int64)
    for c in cores_raw:
        r_v = c["c_v"] // rw
        np.maximum.at(Lmax, (c["w_e"], r_v), c["l_e"] + 1)
    Lmax = np.maximum(Lmax, 1)
    # enforce non-increasing in r (suffix max) for the prefix property
    for w in range(NW):
        for r in range(NRANGE - 2, -1, -1):
            Lmax[w][r] = max(Lmax[w][r], Lmax[w][r + 1])

    # ---- layer-major S layout per window ----
    secL_rel = np.zeros((NW, int(Lmax.max())), np.int64)
    W_l = np.zeros((NW, int(Lmax.max())), np.int64)
    S_w = np.zeros(NW, np.int64)
    for w in range(NW):
        o = 0
        for l in range(int(Lmax[w].max())):
            n_l = int((Lmax[w] > l).sum())
            wl = int(widths[:n_l].sum())
            secL_rel[w][l] = o
            W_l[w][l] = wl
            o += wl
        S_w[w] = even(o)
        assert S_w[w] <= MAXELEMS, f"S window {w} = {S_w[w]} > {MAXELEMS}"
    sm_off = np.r_[0, np.cumsum(S_w)]

    meta = dict(
        nc=nc, csh=csh, sh=sh, npad=npad, cf=cf, dmax=dmax, bpw=bpw,
        CLS=even(CLS), XW=XW, m=m, x0_off=x0_off, x0_woff=x0_woff,
        n_wr=n_wr, xsec_rel=xsec_rel, W_w=W_w, xw_off=xw_off,
        copy_list=copy_list, rw=rw, widths=widths, Lmax=Lmax,
        secL_rel=secL_rel, W_l=W_l, S_w=S_w, sm_off=sm_off,
    )
    per_core = [emit_core_arrays(c, meta) for c in cores_raw]
    return meta, per_core


def emit_core_arrays(c, meta):
    cf = meta["cf"]
    dmax = meta["dmax"]
    x0_off, x0_woff = meta["x0_off"], meta["x0_woff"]
    xsec_rel, W_w = meta["xsec_rel"], meta["W_w"]
    secL_rel = meta["secL_rel"]
    rw = meta["rw"]
    bpw = meta["bpw"]

    # class rank of each source within (p, w, d)
    ck = (c["usrc_p"] * NW + c["win_of_usrc"]) * (dmax + 1) + c["d_src"]
    cls_rank = _cumcount(ck)

    # ls1: gather state -> class layout
    ls1 = np.full((P, cf), -1, np.int16)
    tgt = x0_off[c["win_of_usrc"], c["d_src"]] + cls_rank
    assert tgt.max() < meta["CLS"]
    ls1[c["usrc_p"], c["usrc_q"]] = tgt.astype(np.int16)

    # window-relative class index per source, then per edge
    clsrel_src = (x0_off[c["win_of_usrc"], c["d_src"]]
                  - x0_woff[c["win_of_usrc"]] + cls_rank)
    si = c["src_inv"]
    xpos = xsec_rel[c["w_e"], c["r_in_src"]] + clsrel_src[si]

    # ls2: expanded X window -> transpose blocks
    ls2 = []
    for w in range(NW):
        a2 = np.full((P, int(W_w[w])), -1, np.int16)
        sel = c["w_e"] == w
        t2 = c["b_e"][sel] * P + c["j_e"][sel]
        assert len(t2) == 0 or t2.max() < bpw * P
        a2[c["p_s"][sel], xpos[sel]] = t2.astype(np.int16)
        ls2.append(a2)

    # ls3: transposed blocks -> layer-major S window
    r_v = c["c_v"] // rw
    ls3 = []
    for w in range(NW):
        arr = np.full((P, bpw * P), -1, np.int16)
        sel = c["w_e"] == w
        ipos = c["b_e"][sel] * P + c["p_s"][sel]
        t3 = secL_rel[w, c["l_e"][sel]] + c["c_v"][sel]
        assert len(t3) == 0 or t3.max() < meta["S_w"][w]
        arr[c["j_e"][sel], ipos] = t3.astype(np.int16)
        ls3.append(arr)

    return dict(ls1=ls1, ls2=ls2, ls3=ls3)


# ──────────────────────────────────────────────────────────────────────
# numpy emulation of one round (validation)
# ──────────────────────────────────────────────────────────────────────

def _emu_ls(data, idx, num_elems):
    Pp, n = idx.shape
    assert data.shape[0] == Pp and data.shape[1] >= n
    out = np.zeros((Pp, num_elems), data.dtype)
    for p in range(Pp):
        ii = idx[p].astype(np.int64)
        valid = ii >= 0
        assert len(np.unique(ii[valid])) == valid.sum(), "dup idx"
        out[p, ii[valid]] = data[p, :n][valid]
    return out


def emulate_round(w_full, meta, arrs, fdtype=np.float32):
    """w_full: [npad] pre-scaled state (device order). Returns u [SEGS,csh]
    = unscaled scatter-add for this core."""
    cf, csh = meta["cf"], meta["csh"]
    CLS, XW = meta["CLS"], meta["XW"]
    bpw = meta["bpw"]
    W_w, xw_off = meta["W_w"], meta["xw_off"]
    S_w, sm_off = meta["S_w"], meta["sm_off"]
    secL_rel, W_l, Lmax = meta["secL_rel"], meta["W_l"], meta["Lmax"]

    state = w_full.reshape(P, cf).astype(fdtype)
    x0 = _emu_ls(state, arrs["ls1"], CLS)

    X = np.zeros((P, XW), fdtype)
    for (w, dst, n) in meta["copy_list"]:
        X[:, xw_off[w] + dst: xw_off[w] + dst + n] = \
            x0[:, meta["x0_woff"][w]: meta["x0_woff"][w] + n]

    F = NW * bpw * P
    x2 = np.zeros((P, F), fdtype)
    for w in range(NW):
        o = _emu_ls(X[:, xw_off[w]:xw_off[w] + W_w[w]], arrs["ls2"][w],
                    bpw * P)
        x2[:, w * bpw * P:(w + 1) * bpw * P] = o

    xt = np.zeros((P, F), fdtype)
    for b in range(NW * bpw):
        xt[:, b * P:(b + 1) * P] = x2[:, b * P:(b + 1) * P].T

    y = np.zeros((NW, P, csh), fdtype)
    for w in range(NW):
        sm = _emu_ls(xt[:, w * bpw * P:(w + 1) * bpw * P], arrs["ls3"][w],
                     int(S_w[w]))
        for l in range(int(Lmax[w].max())):
            wl = int(W_l[w][l])
            y[w][:, :wl] += sm[:, int(secL_rel[w][l]):int(secL_rel[w][l]) + wl]

    u = np.zeros((SEGS, csh), fdtype)
    for w in range(NW):
        u += y[w].reshape(SEGS, R4, csh).sum(axis=1)
    return u


# ──────────────────────────────────────────────────────────────────────
# preprocess / inputs
# ──────────────────────────────────────────────────────────────────────

def next_pow2(x):
    p = 1
    while p < x:
        p *= 2
    return p


def preprocess(x, edge_index, batch, nc_count=8, n_graphs=128):
    n_nodes = x.shape[0]
    row = np.asarray(edge_index[0], np.int64)
    col = np.asarray(edge_index[1], np.int64)
    batch = np.asarray(batch, np.int64)

    csh, sh, npad, cf = build_layout(n_nodes, nc_count)
    deg = np.bincount(col, minlength=npad).astype(np.int64)
    flat, layout = relabel(deg, n_nodes, nc_count)
    re, ve = flat[row], flat[col]

    meta = per_core = None
    for bpw in (6, 7, 8):
        try:
            cores_raw = []
            for c in range(nc_count):
                mm = (ve // sh) == c
                cores_raw.append(build_core(c, re[mm], ve[mm], layout, bpw))
            meta, per_core = finalize_cores(cores_raw, layout, bpw)
            break
        except (AssertionError, RuntimeError):
            continue
    assert meta is not None, "routing build failed for all bpw"

    # device-order node arrays
    inv = np.empty(npad, np.int64)
    inv[flat] = np.arange(npad)
    deg_dev = deg[inv].astype(np.float32)
    batch_dev = np.full(npad, -1, np.int64)
    batch_dev[flat[:n_nodes]] = batch[:n_nodes]

    # ---- pooling structures ----
    g0 = np.zeros(nc_count, np.int64)
    ngl = np.zeros(nc_count, np.int64)
    wg_max = 0
    for c in range(nc_count):
        bd = batch_dev[c * sh:(c + 1) * sh]
        real = bd >= 0
        gmin, gmax = (int(bd[real].min()), int(bd[real].max())) \
            if real.any() else (0, 0)
        g0[c], ngl[c] = gmin, gmax - gmin + 1
        fin = np.arange(sh)
        s = fin // csh
        cnt = np.zeros((SEGS, int(ngl[c])), np.int64)
        np.add.at(cnt, (s[real], bd[real] - gmin), 1)
        wg_max = max(wg_max, int(cnt.max()))
    NGLP = int(ngl.max())
    WGP = next_pow2(wg_max)
    GPH = max(1, min(MAXELEMS // WGP, NGLP))
    NPH = cdiv(NGLP, GPH)
    PHALF = GPH * WGP
    assert NPH * GPH <= 32, f"pool pad {NPH * GPH} > 32"

    pool_idx = []
    gms = [np.zeros((P, P), np.float32) for _ in range(2)]
    for c in range(nc_count):
        bd = batch_dev[c * sh:(c + 1) * sh]
        fin = np.arange(sh)
        s, cc = fin // csh, fin % csh
        lg = bd - g0[c]
        arrs = []
        rank = np.zeros(sh, np.int64)
        real = bd >= 0
        key = s * 4096 + lg
        rank[real] = _cumcount(key[real])
        assert rank.max() < WGP
        for h in range(NPH):
            a = np.full((SEGS, csh), -1, np.int16)
            selh = real & (lg >= h * GPH) & (lg < (h + 1) * GPH)
            tgt = (lg[selh] - h * GPH) * WGP + rank[selh]
            assert len(tgt) == 0 or tgt.max() < PHALF
            a[s[selh], cc[selh]] = tgt.astype(np.int16)
            arrs.append(a)
        pool_idx.append(arrs)
        for li in range(NPH * GPH):
            g = g0[c] + li
            if li < int(ngl[c]) and g < n_graphs:
                fp = c * 32 + li
                gms[fp % 2][fp // 2, g] = 1.0

    pool_meta = dict(NGLP=NPH * GPH, NGH=GPH, WGP=WGP, NPH=NPH, PHALF=PHALF,
                     g0=g0)
    return meta, per_core, pool_meta, pool_idx, gms, flat, deg_dev, layout


def _xtpad(xsh):
    sh = xsh.shape[0]
    padsh = cdiv(sh, P) * P
    out = np.zeros((xsh.shape[1], padsh), np.float32)
    out[:, :sh] = xsh.T
    return out.astype(BF16)


def _degq(degsh):
    sh = len(degsh)
    nblk = cdiv(sh, P)
    pad = np.zeros(nblk * P, np.float32)
    pad[:sh] = degsh
    return np.ascontiguousarray(pad.reshape(nblk, P).T)


def make_inputs(meta, pool_meta, per_core, pool_idx, gms, flat, deg_dev,
                x, weights, n_max, n_graphs=128):
    csh, sh, npad, cf = meta["csh"], meta["sh"], meta["npad"], meta["cf"]
    nc_count = meta["nc"]
    n_nodes = x.shape[0]

    xdev = np.zeros((npad, x.shape[1]), np.float32)
    xdev[flat[:n_nodes]] = x
    in_maps = []
    for c in range(nc_count):
        im = dict(
            xT=_xtpad(xdev[c * sh:(c + 1) * sh]),
            degs=deg_dev[c * sh:(c + 1) * sh].reshape(SEGS, csh),
            degq=_degq(deg_dev[c * sh:(c + 1) * sh]),
            ls1=per_core[c]["ls1"],
            w1t=np.ascontiguousarray(weights["W1"].T),
            w2t=np.ascontiguousarray(weights["W2"].T),
            w3t=np.ascontiguousarray(weights["W3"].T),
            w4t=np.ascontiguousarray(weights["W4"].T),
            wl=np.ascontiguousarray(weights["Wl"]).reshape(64, 1),
            bl=np.asarray(weights["bl"], np.float32).reshape(1, 1),
            invn=np.asarray([[1.0 / np.float32(n_max)]], np.float32),
            selb=np.repeat(np.eye(SEGS, dtype=np.float32),
                           R4, axis=0).astype(BF16),
            identb=np.eye(P, dtype=np.float32).astype(BF16),
            ones16=np.ones((SEGS, 1), np.float32),
            gm0=gms[0], gm1=gms[1],
        )
        for k in range(1, 5):
            im[f"b{k}"] = np.asarray(weights[f"b{k}"], np.float32).reshape(64, 1)
        for w in range(NW):
            im[f"ls2_{w}"] = per_core[c]["ls2"][w]
            im[f"ls3_{w}"] = per_core[c]["ls3"][w]
        for h in range(pool_meta["NPH"]):
            im[f"pool_{h}"] = pool_idx[c][h]
        in_maps.append(im)
    return in_maps


def reference_numpy(x, edge_index, batch, weights, n_graphs=128):
    row = np.asarray(edge_index[0]); col = np.asarray(edge_index[1])
    N = x.shape[0]
    deg = np.bincount(col, minlength=N).astype(np.float64)
    dinv = np.where(deg > 0, deg ** -0.5, 0.0)
    norm = dinv[row] * dinv[col]
    h = x.astype(np.float64)
    for k in range(1, 5):
        W = weights[f"W{k}"]
        b = weights[f"b{k}"]
        hw = h @ W
        msg = norm[:, None] * hw[row]
        out = np.zeros((N, hw.shape[1]))
        np.add.at(out, col, msg)
        h = out + b
    sums = np.zeros((n_graphs, h.shape[1]))
    np.add.at(sums, batch, h)
    counts = np.bincount(batch, minlength=n_graphs)
    pooled = sums / counts.max()
    return (pooled @ weights["Wl"] + weights["bl"]).astype(np.float32)


# ──────────────────────────────────────────────────────────────────────
# device kernel
# ──────────────────────────────────────────────────────────────────────
from contextlib import ExitStack

import concourse.bass as bass
import concourse.tile as tile
from concourse import bacc, mybir

FP32 = mybir.dt.float32
BF16D = mybir.dt.bfloat16
I16 = mybir.dt.int16
AT = mybir.ActivationFunctionType
OP = mybir.AluOpType


def build_kernel(meta, pool_meta, n_graphs=128):
    csh, sh, npad, cf = meta["csh"], meta["sh"], meta["npad"], meta["cf"]
    CLS, XW = meta["CLS"], meta["XW"]
    bpw = meta["bpw"]
    W_w, xw_off = meta["W_w"], meta["xw_off"]
    S_w, sm_off = meta["S_w"], meta["sm_off"]
    secL_rel, W_l, Lmax = meta["secL_rel"], meta["W_l"], meta["Lmax"]
    NB = NW * bpw                       # total transpose blocks
    NGLP, WGP = pool_meta["NGLP"], pool_meta["WGP"]
    PHALF = pool_meta["PHALF"]
    NPH = pool_meta["NPH"]
    core_ids = list(range(meta["nc"]))
    cuts = [(0, min(512, csh))]
    if csh > 512:
        cuts.append((512, csh))

    nc = bacc.Bacc("TRN2", target_bir_lowering=False, debug=False,
                   num_devices=meta["nc"])

    def din(name, shape, dt=FP32):
        return nc.declare_dram_parameter(name, list(shape), dt, isOutput=False)

    # ---- inputs ----
    NBLK = cdiv(sh, P)                  # 128-node blocks for the s0 matvec
    lastw = sh - (NBLK - 1) * P
    xT_in = din("xT", [P, NBLK * P], BF16D)
    degs_in = din("degs", [SEGS, csh])
    degq_in = din("degq", [P, NBLK])
    ls1_in = din("ls1", [P, cf], I16)
    ls2_in = [din(f"ls2_{w}", [P, int(W_w[w])], I16) for w in range(NW)]
    ls3_in = [din(f"ls3_{w}", [P, bpw * P], I16) for w in range(NW)]
    pool_in = [din(f"pool_{h}", [SEGS, csh], I16) for h in range(NPH)]
    w1t_in = din("w1t", [64, 128])
    w2t_in = din("w2t", [64, 64])
    w3t_in = din("w3t", [64, 64])
    w4t_in = din("w4t", [64, 64])
    wl_in = din("wl", [64, 1])
    b_in = [din(f"b{k}", [64, 1]) for k in range(1, 5)]
    bl_in = din("bl", [1, 1])
    invn_in = din("invn", [1, 1])
    selb_in = din("selb", [P, SEGS], BF16D)
    identb_in = din("identb", [P, P], BF16D)
    ones16_in = din("ones16", [SEGS, 1])
    gm_in = [din(f"gm{i}", [P, P]) for i in range(2)]
    out_ext = nc.declare_dram_parameter("out", [n_graphs], FP32, isOutput=True)

    # ---- internal DRAM ----
    sh_dram = nc.dram_tensor("sh_dram", [sh], BF16D)
    full_dram = nc.dram_tensor("full_dram", [npad], BF16D, addr_space="Shared")
    part_dram = nc.dram_tensor("part_dram", [32], FP32)
    warm_in = nc.dram_tensor("warm_in", [32], FP32)
    warm_out = nc.dram_tensor("warm_out", [256], FP32, addr_space="Shared")
    partall_dram = nc.dram_tensor("partall_dram", [256], FP32,
                                  addr_space="Shared")

    with tile.TileContext(nc) as tc:
        with ExitStack() as ctx:
            pool = ctx.enter_context(tc.tile_pool(name="p", bufs=1))
            tp = ctx.enter_context(tc.tile_pool(name="tp", bufs=2,
                                                space="PSUM"))
            up = ctx.enter_context(tc.tile_pool(name="up", bufs=2,
                                                space="PSUM"))

            # persistent tiles
            state = pool.tile([P, cf], BF16D)
            x0 = pool.tile([P, CLS], BF16D)
            X = pool.tile([P, XW], BF16D)
            x2 = pool.tile([P, NB * P], BF16D)
            xt = pool.tile([P, NB * P], BF16D)
            sm = pool.tile([P, int(sm_off[-1])], BF16D)
            yw = [pool.tile([P, csh], BF16D, name=f"y{w}") for w in range(NW)]
            dinvs = pool.tile([SEGS, csh], FP32)
            dinvs2 = pool.tile([SEGS, csh], FP32)
            bd = [pool.tile([SEGS, csh], FP32, name=f"bd{k}")
                  for k in range(NW)]
            ts1 = pool.tile([SEGS, csh], FP32)
            wout = pool.tile([SEGS, csh], BF16D)
            s4 = pool.tile([SEGS, csh], FP32)
            degs = pool.tile([SEGS, csh], FP32)
            tmp16 = pool.tile([SEGS, csh], FP32)
            degq = pool.tile([P, NBLK], FP32)
            dinvq = pool.tile([P, NBLK], FP32)
            tmpq = pool.tile([P, NBLK], FP32)
            wout0 = pool.tile([P, NBLK], BF16D)
            selb = pool.tile([P, SEGS], BF16D)
            identb = pool.tile([P, P], BF16D)
            ones16 = pool.tile([SEGS, 1], FP32)
            gm = [pool.tile([P, P], FP32, name=f"gm{i}") for i in range(2)]
            ls1 = pool.tile([P, cf], I16)
            ls2 = [pool.tile([P, int(W_w[w])], I16, name=f"ls2t{w}")
                   for w in range(NW)]
            ls3 = [pool.tile([P, bpw * P], I16, name=f"ls3t{w}")
                   for w in range(NW)]
            plidx = [pool.tile([SEGS, csh], I16, name=f"plidx{h}")
                     for h in range(NPH)]
            poolsrc = pool.tile([SEGS, csh], BF16D)
            poolbuf = pool.tile([SEGS, NPH * PHALF], BF16D)
            poolf32 = pool.tile([SEGS, NPH * PHALF], FP32)
            p16 = pool.tile([SEGS, NGLP], FP32)
            part_sb = pool.tile([1, 32], FP32)
            partall = pool.tile([P, 2], FP32)
            outrow = pool.tile([1, n_graphs], FP32)
            wts = {
                "w1t": pool.tile([64, 128], FP32, name="w1t_t"),
                "w2t": pool.tile([64, 64], FP32, name="w2t_t"),
                "w3t": pool.tile([64, 64], FP32, name="w3t_t"),
                "w4t": pool.tile([64, 64], FP32, name="w4t_t"),
                "wl": pool.tile([64, 1], FP32, name="wl_t"),
            }
            bs = [pool.tile([64, 1], FP32, name=f"bs{k}") for k in range(4)]
            blt = pool.tile([1, 1], FP32)
            invn = pool.tile([1, 1], FP32)
            cvec = {
                "c3": pool.tile([64, 1], FP32, name="c3t"),
                "c2": pool.tile([64, 1], FP32, name="c2t"),
                "c1": pool.tile([64, 1], FP32, name="c1t"),
                "c0": pool.tile([128, 1], FP32, name="c0t"),
            }
            c0b = pool.tile([128, 1], BF16D)
            betas = pool.tile([1, 4], FP32)
            ones116 = pool.tile([1, 16], FP32)
            betas16 = pool.tile([SEGS, 4], FP32)

            # ---- loads ----
            warmsb = pool.tile([1, 32], FP32)
            nc.vector.memset(warmsb[:], 0.0)
            nc.sync.dma_start(warm_in[:].rearrange("(a b) -> a b", a=1),
                              warmsb[:])
            nc.gpsimd.collective_compute(
                "AllGather", OP.bypass, replica_groups=[core_ids],
                ins=[warm_in[:]], outs=[warm_out[:]],
            )
            nc.sync.dma_start(ls1[:], ls1_in[:])
            for w in range(NW):
                nc.sync.dma_start(ls2[w][:], ls2_in[w][:])
                nc.sync.dma_start(ls3[w][:], ls3_in[w][:])
            for h in range(NPH):
                nc.sync.dma_start(plidx[h][:], pool_in[h][:])
            nc.sync.dma_start(selb[:], selb_in[:])
            nc.sync.dma_start(identb[:], identb_in[:])
            nc.sync.dma_start(ones16[:], ones16_in[:])
            for i in range(2):
                nc.sync.dma_start(gm[i][:], gm_in[i][:])
            for k, t in wts.items():
                nc.sync.dma_start(t[:], {"w1t": w1t_in, "w2t": w2t_in,
                                         "w3t": w3t_in, "w4t": w4t_in,
                                         "wl": wl_in}[k][:])
            for k in range(4):
                nc.sync.dma_start(bs[k][:], b_in[k][:])
            nc.sync.dma_start(blt[:], bl_in[:])
            nc.sync.dma_start(invn[:], invn_in[:])
            nc.sync.dma_start(degs[:], degs_in[:])
            nc.sync.dma_start(degq[:], degq_in[:])

            # ---- dinv = rsqrt(deg + (deg==0)) * (deg>0) ----
            def make_dinv(dst, deg_t, tmp):
                nc.vector.tensor_scalar(tmp[:], deg_t[:], 0.0, None,
                                        OP.is_equal)
                nc.vector.tensor_tensor(tmp[:], tmp[:], deg_t[:], OP.add)
                nc.scalar.activation(tmp[:], tmp[:], AT.Sqrt)
                nc.vector.reciprocal(tmp[:], tmp[:])
                nc.vector.tensor_scalar(dst[:], deg_t[:], 0.0, None, OP.is_gt)
                nc.vector.tensor_tensor(dst[:], dst[:], tmp[:], OP.mult)

            make_dinv(dinvs, degs, tmp16)
            make_dinv(dinvq, degq, tmpq)
            nc.vector.tensor_tensor(dinvs2[:], dinvs[:], dinvs[:], OP.mult)

            # ---- c chain + betas ----
            pc = tp.tile([128, 4], FP32, tag="ops", bufs=1)
            nc.tensor.matmul(pc[0:64, 0:1], wts["w4t"][:], wts["wl"][:],
                             start=True, stop=True)
            nc.vector.tensor_copy(cvec["c3"][:], pc[0:64, 0:1])
            nc.tensor.matmul(pc[0:64, 1:2], wts["w3t"][:], cvec["c3"][:],
                             start=True, stop=True)
            nc.vector.tensor_copy(cvec["c2"][:], pc[0:64, 1:2])
            nc.tensor.matmul(pc[0:64, 2:3], wts["w2t"][:], cvec["c2"][:],
                             start=True, stop=True)
            nc.vector.tensor_copy(cvec["c1"][:], pc[0:64, 2:3])
            nc.tensor.matmul(pc[0:128, 3:4], wts["w1t"][:], cvec["c1"][:],
                             start=True, stop=True)
            nc.vector.tensor_copy(cvec["c0"][:], pc[0:128, 3:4])
            nc.vector.tensor_copy(c0b[:], cvec["c0"][:])
            pb = tp.tile([1, 4], FP32, tag="ops", bufs=1)
            for k, cn in enumerate(["c1", "c2", "c3"]):
                nc.tensor.matmul(pb[0:1, k:k + 1], bs[k][:], cvec[cn][:],
                                 start=True, stop=True)
            nc.tensor.matmul(pb[0:1, 3:4], bs[3][:], wts["wl"][:],
                             start=True, stop=True)
            nc.vector.tensor_copy(betas[:], pb[:])
            nc.vector.memset(ones116[:], 1.0)
            pbb = tp.tile([SEGS, 4], FP32, tag="ops", bufs=1)
            nc.tensor.matmul(pbb[:], ones116[:], betas[:], start=True,
                             stop=True)
            nc.vector.tensor_copy(betas16[:], pbb[:])
            # bd[k] = dinvs * beta_k   (k = 0..2 for rounds 0..2)
            for k in range(NW):
                nc.vector.tensor_scalar(bd[k][:], dinvs[:],
                                        betas16[:, k:k + 1], None, OP.mult)

            # ---- s0 = x @ c0: stationary 128-node blocks, col k per block ----
            pu0 = up.tile([P, NBLK], FP32, tag="pu0", bufs=1, name="pu_s0")
            BPC = 25                     # blocks per xq chunk
            for q in range(cdiv(NBLK, BPC)):
                b0_, b1_ = q * BPC, min((q + 1) * BPC, NBLK)
                wq = (b1_ - b0_) * P
                xq = pool.tile([P, BPC * P], BF16D, tag="xq", bufs=2,
                               name=f"xq{q}")
                nc.sync.dma_start(xq[:, 0:wq],
                                  xT_in[:, b0_ * P:b0_ * P + wq])
                for b in range(b0_, b1_):
                    nc.tensor.matmul(
                        pu0[:, b:b + 1],
                        xq[:, (b - b0_) * P:(b - b0_ + 1) * P],
                        c0b[:], start=True, stop=True)
            nc.vector.tensor_tensor(wout0[:], pu0[:], dinvq[:], OP.mult)
            # transpose to block-major [NBLK, 128] for a contiguous DMA
            pt0 = tp.tile([P, P], BF16D, tag="ptr", name="pt_s0")
            nc.tensor.transpose(pt0[0:NBLK, :], wout0[:], identb[:])
            ws0 = pool.tile([P, P], BF16D)
            nc.vector.tensor_copy(ws0[0:NBLK, :], pt0[0:NBLK, :])

            # ---- rounds ----
            for rnd in range(4):
                if rnd == 0:
                    nc.sync.dma_start(
                        sh_dram[0:(NBLK - 1) * P].rearrange(
                            "(k m) -> k m", k=NBLK - 1),
                        ws0[0:NBLK - 1, :])
                    nc.sync.dma_start(
                        sh_dram[(NBLK - 1) * P:sh].rearrange(
                            "(a b) -> a b", a=1),
                        ws0[NBLK - 1:NBLK, 0:lastw])
                else:
                    nc.sync.dma_start(
                        sh_dram[:].rearrange("(a b) -> a b", a=SEGS), wout[:])
                nc.gpsimd.collective_compute(
                    "AllGather", OP.bypass, replica_groups=[core_ids],
                    ins=[sh_dram[:]], outs=[full_dram[:]],
                )
                nc.sync.dma_start(
                    state[:], full_dram[:].rearrange("(p c) -> p c", p=P))

                nc.gpsimd.local_scatter(
                    x0[:], state[:], ls1[:],
                    channels=P, num_elems=CLS, num_idxs=cf)
                for (w, dst, n) in meta["copy_list"]:
                    nc.vector.tensor_copy(
                        X[:, int(xw_off[w]) + dst:int(xw_off[w]) + dst + n],
                        x0[:, int(meta["x0_woff"][w]):
                           int(meta["x0_woff"][w]) + n])

                def emit_ls2(w):
                    nc.gpsimd.local_scatter(
                        x2[:, w * bpw * P:(w + 1) * bpw * P],
                        X[:, int(xw_off[w]):int(xw_off[w]) + int(W_w[w])],
                        ls2[w][:], channels=P, num_elems=bpw * P,
                        num_idxs=int(W_w[w]))

                def emit_transp(w):
                    blocks = list(range(w * bpw, (w + 1) * bpw))
                    for g0_ in range(0, len(blocks), 4):
                        grp = blocks[g0_:g0_ + 4]
                        pt = tp.tile([P, 4 * P], BF16D, tag="ptr",
                                     name=f"pt{rnd}_{w}_{g0_}")
                        for k, b in enumerate(grp):
                            nc.tensor.transpose(pt[:, k * P:(k + 1) * P],
                                                x2[:, b * P:(b + 1) * P],
                                                identb[:])
                        nc.vector.tensor_copy(
                            xt[:, grp[0] * P:(grp[-1] + 1) * P],
                            pt[:, 0:len(grp) * P])

                def emit_ls3(w):
                    nc.gpsimd.local_scatter(
                        sm[:, int(sm_off[w]):int(sm_off[w]) + int(S_w[w])],
                        xt[:, w * bpw * P:(w + 1) * bpw * P],
                        ls3[w][:], channels=P, num_elems=int(S_w[w]),
                        num_idxs=bpw * P)

                # gpsimd order: ls2_0, ls2_1, ls3_0, ls2_2, ls3_1, ls3_2
                emit_ls2(0)
                emit_ls2(1)
                emit_transp(0)
                emit_ls3(0)
                emit_ls2(2)
                emit_transp(1)
                emit_ls3(1)
                emit_transp(2)
                emit_ls3(2)

                # layer sums into y_w (bf16)
                for w in range(NW):
                    base = int(sm_off[w])
                    nc.vector.tensor_copy(
                        yw[w][:, 0:int(W_l[w][0])],
                        sm[:, base:base + int(W_l[w][0])])
                    for l in range(1, int(Lmax[w].max())):
                        wl_ = int(W_l[w][l])
                        o = base + int(secL_rel[w][l])
                        nc.vector.tensor_tensor(
                            yw[w][:, 0:wl_], yw[w][:, 0:wl_],
                            sm[:, o:o + wl_], OP.add)

                # segment reduction: psum-accumulated sel matmuls
                pu = up.tile([SEGS, csh], FP32, tag="pu", name=f"pu{rnd}")
                for (a, b2) in cuts:
                    for w in range(NW):
                        nc.tensor.matmul(pu[:, a:b2], selb[:],
                                         yw[w][:, a:b2],
                                         start=(w == 0), stop=(w == NW - 1))

                if rnd < 3:
                    nc.vector.tensor_tensor(ts1[:], pu[:], dinvs2[:], OP.mult)
                    nc.vector.tensor_tensor(wout[:], ts1[:], bd[rnd][:],
                                            OP.add)
                else:
                    nc.vector.tensor_tensor(ts1[:], pu[:], dinvs[:], OP.mult)
                    nc.vector.tensor_scalar(s4[:], ts1[:],
                                            betas16[:, 3:4], None, OP.add)

            # ---- pooling ----
            nc.vector.tensor_copy(poolsrc[:], s4[:])
            for h in range(NPH):
                nc.gpsimd.local_scatter(
                    poolbuf[:, h * PHALF:(h + 1) * PHALF],
                    poolsrc[:], plidx[h][:],
                    channels=SEGS, num_elems=PHALF, num_idxs=csh)
            nc.vector.tensor_copy(poolf32[:], poolbuf[:])
            wgp = WGP
            a = poolf32[:].rearrange("s (g t) -> s g t", t=WGP)
            while wgp > 1:
                hw = wgp // 2
                nc.vector.tensor_tensor(
                    a[:, :, 0:hw], a[:, :, 0:hw], a[:, :, hw:wgp], OP.add)
                wgp = hw
            nc.vector.tensor_copy(
                p16[:], a[:, :, 0:1].rearrange("s g t -> s (g t)"))
            pp = tp.tile([1, NGLP], FP32, tag="ops", bufs=1)
            nc.tensor.matmul(pp[:], ones16[:], p16[:], start=True, stop=True)
            nc.vector.memset(part_sb[:], 0.0)
            nc.vector.tensor_copy(part_sb[:, 0:NGLP], pp[:])
            nc.sync.dma_start(part_dram[:].rearrange("(a b) -> a b", a=1),
                              part_sb[:])
            nc.gpsimd.collective_compute(
                "AllGather", OP.bypass, replica_groups=[core_ids],
                ins=[part_dram[:]], outs=[partall_dram[:]],
            )
            nc.sync.dma_start(partall[:],
                              partall_dram[:].rearrange("(p c) -> p c", p=P))
            po_ = tp.tile([1, n_graphs], FP32, tag="ops", bufs=1)
            nc.tensor.matmul(po_[:], partall[:, 0:1], gm[0][:],
                             start=True, stop=False)
            nc.tensor.matmul(po_[:], partall[:, 1:2], gm[1][:],
                             start=False, stop=True)
            nc.vector.tensor_copy(outrow[:], po_[:])
            nc.vector.tensor_scalar(outrow[:], outrow[:], invn[0:1, 0:1],
                                    None, OP.mult)
            nc.vector.tensor_scalar(outrow[:], outrow[:], blt[0:1, 0:1],
                                    None, OP.add)
            nc.sync.dma_start(out_ext[:].rearrange("(a b) -> a b", a=1),
                              outrow[:])
    return nc


# ─── entry point ───

def kernel(x, edge_index, batch, W1, b1, W2, b2, W3, b3, W4, b4, Wl, bl):
    from concourse.bass_utils import run_bass_kernel_spmd

    x = np.asarray(x, np.float32)
    edge_index = np.asarray(edge_index)
    batch = np.asarray(batch)
    weights = dict(W1=np.asarray(W1, np.float32), W2=np.asarray(W2, np.float32),
                   W3=np.asarray(W3, np.float32), W4=np.asarray(W4, np.float32),
                   Wl=np.asarray(Wl, np.float32),
                   b1=np.asarray(b1, np.float32), b2=np.asarray(b2, np.float32),
                   b3=np.asarray(b3, np.float32), b4=np.asarray(b4, np.float32),
                   bl=np.asarray(bl, np.float32))
    n_graphs = 128

    meta, per_core, pool_meta, pool_idx, gms, flat, deg_dev, layout = \
        preprocess(x, edge_index, batch, 8, n_graphs)
    n_max = int(np.bincount(np.asarray(batch, np.int64),
                            minlength=n_graphs).max())
    in_maps = make_inputs(meta, pool_meta, per_core, pool_idx, gms, flat,
                          deg_dev, x, weights, n_max, n_graphs)
    nc = build_kernel(meta, pool_meta, n_graphs)
    nc.finalize()
    res = run_bass_kernel_spmd(nc, in_maps, core_ids=list(range(8)),
                               trace=False)
    return res.results[0]["out"].reshape(n_graphs, 1).astype(np.float32)


# revision 18
# speedup vs baseline: 1.7376x; 1.0617x over previous
"""Trainium2 kernel for nn_GCNRegression: linear-GCN scalar collapse, bf16.

The model is linear (no activation), so 4 GCN layers + mean-pool + linear
head collapse exactly to scalar propagation through the graph:
    c0 = W1 @ W2 @ W3 @ W4 @ Wl;  s0 = x @ c0
    s_k = dinv * (Adj^T @ (dinv * s_{k-1})) + b_k . c_k
    out[g] = sum_{v in g} s4[v] / n_max + bl
8 NeuronCores, destination-sharded edges. Per round: AllGather the
dinv-pre-scaled bf16 state, per-partition local_scatter routing (gather
classes -> expand by out-degree -> scatter into transpose blocks), PE
transposes, local_scatter into a layer-major segment layout, DVE layer
sums, one PSUM-accumulated sel matmul, scale + bias.  All index arrays
are host-precomputed from the edge list; values move as bf16 (single
int16 slot per value in every local_scatter).
"""

import sys

sys.path.insert(0, "/opt/trn_rl_repo")

import numpy as np
import ml_dtypes

BF16 = ml_dtypes.bfloat16

P = 128          # partitions
SEGS = 16        # shard rows (psum partitions)
R4 = P // SEGS   # 8 partition rows per segment
NW = 2           # windows (= LS2/LS3 call count)
NRANGE = 16      # column ranges for layer caps
MAXELEMS = 2046  # local_scatter out-region limit (num_elems*32 < 2^16)


def cdiv(a, b):
    return (a + b - 1) // b


def even(x):
    return x + (x & 1)


def _cumcount(keys):
    """Rank of each element within its key group (stable, array order)."""
    order = np.argsort(keys, kind="stable")
    sk = keys[order]
    grp_start = np.r_[0, np.flatnonzero(sk[1:] != sk[:-1]) + 1]
    sizes = np.diff(np.r_[grp_start, len(keys)])
    cum = np.arange(len(keys)) - np.repeat(grp_start, sizes)
    out = np.empty(len(keys), np.int64)
    out[order] = cum
    return out


def build_layout(n_nodes, nc):
    csh = cdiv(n_nodes, nc * SEGS)
    sh = SEGS * csh
    npad = nc * sh
    cf = npad // P
    return csh, sh, npad, cf


def relabel(deg, n_nodes, nc):
    """Shard by original id; within shard sort by in-degree desc; lay
    column-major into [SEGS, CSH]. Returns flat[] over padded ids."""
    csh, sh, npad, cf = build_layout(n_nodes, nc)
    flat = np.empty(npad, np.int64)
    for c in range(nc):
        ids = np.arange(c * sh, (c + 1) * sh)
        order = np.argsort(-deg[ids], kind="stable")
        t = np.empty(len(ids), np.int64)
        t[order] = np.arange(len(ids))
        s, cc = t % SEGS, t // SEGS
        flat[ids] = c * sh + s * csh + cc
    return flat, (csh, sh, npad, cf)


def build_core(core, re, ve, layout, bpw):
    """Per-core routing. re/ve: device-flat src/dst positions."""
    csh, sh, npad, cf = layout
    E = len(re)
    p_s = re // cf
    q_s = re % cf
    fin = ve - core * sh
    s_v = fin // csh
    c_v = fin % csh

    # ---- window per source: per (partition, count-desc) round robin ----
    usrc, src_inv, src_cnt = np.unique(re, return_inverse=True,
                                       return_counts=True)
    usrc_p = usrc // cf
    so = np.lexsort((-src_cnt, usrc_p))
    rank_in_p = _cumcount(usrc_p[so])
    win_of_usrc = np.empty(len(usrc), np.int64)
    win_of_usrc[so] = rank_in_p % NW
    w_e = win_of_usrc[src_inv]

    # ---- initial j: balanced round-robin within (v, w) ----
    jr = _cumcount(fin * NW + w_e) % R4
    j_e = s_v * R4 + jr

    # ---- repair (p_s, w, j) column loads to <= bpw ----
    def psj(j):
        return (p_s * NW + w_e) * P + j

    cnt = np.bincount(psj(j_e), minlength=P * NW * P)
    vwj_key_all = (fin * NW + w_e) * P
    vwj = {}
    vk = vwj_key_all + j_e
    uk, uc = np.unique(vk, return_counts=True)
    vwj = dict(zip(uk.tolist(), uc.tolist()))
    for _try in range(400):
        rank = _cumcount(psj(j_e))
        move = np.flatnonzero(rank >= bpw)
        if len(move) == 0:
            break
        for i in move:
            base = s_v[i] * R4
            pw = (p_s[i] * NW + w_e[i]) * P
            best = None
            for r in range(R4):
                jv = base + r
                if jv == j_e[i]:
                    continue
                ld = cnt[pw + jv]
                if ld >= bpw:
                    continue
                nv = vwj.get(vwj_key_all[i] + jv, 0)
                key = (nv, ld)
                if best is None or key < best[0]:
                    best = (key, jv)
            if best is None:
                ld0 = [cnt[pw + base + r] for r in range(R4)]
                jv = base + int(np.argmin(ld0))
            else:
                jv = best[1]
            vwj[vwj_key_all[i] + j_e[i]] -= 1
            cnt[pw + j_e[i]] -= 1
            j_e[i] = jv
            cnt[pw + jv] += 1
            vwj[vwj_key_all[i] + jv] = vwj.get(vwj_key_all[i] + jv, 0) + 1
    else:
        raise RuntimeError("j balance failed")
    assert np.bincount(psj(j_e), minlength=P * NW * P).max() <= bpw

    b_e = _cumcount(psj(j_e))                       # block rank in [0,bpw)
    l_e = _cumcount((fin * NW + w_e) * P + j_e)     # layer rank per (v,w,j)
    r_in_src = _cumcount(src_inv)                   # edge rank within source

    return dict(
        E=E, p_s=p_s, q_s=q_s, s_v=s_v, c_v=c_v, w_e=w_e, j_e=j_e,
        b_e=b_e, l_e=l_e, r_in_src=r_in_src,
        usrc=usrc, usrc_p=usrc_p, usrc_q=usrc % cf, d_src=src_cnt,
        win_of_usrc=win_of_usrc, src_inv=src_inv,
    )


def finalize_cores(cores_raw, layout, bpw):
    csh, sh, npad, cf = layout
    nc = len(cores_raw)
    dmax = max(int(c["d_src"].max()) for c in cores_raw)

    # ---- class sizes m[w][d]: max over (core, partition) ----
    m = np.zeros((NW, dmax + 1), np.int64)
    for c in cores_raw:
        cnt = np.zeros((P, NW, dmax + 1), np.int64)
        np.add.at(cnt, (c["usrc_p"], c["win_of_usrc"], c["d_src"]), 1)
        m = np.maximum(m, cnt.max(axis=0))
    m[:, 0] = 0

    # ---- class layout: per window, d DESC ----
    x0_off = np.zeros((NW, dmax + 1), np.int64)
    x0_woff = np.zeros(NW + 1, np.int64)
    o0 = 0
    for w in range(NW):
        x0_woff[w] = o0
        for d in range(dmax, 0, -1):
            if m[w][d] == 0:
                continue
            x0_off[w][d] = o0
            o0 += int(m[w][d])
    x0_woff[NW] = o0
    CLS = int(o0)
    assert even(CLS) <= MAXELEMS, f"CLS={CLS}"

    # ---- expanded X layout: per window, sections r=0..dmax-1 ----
    # section r holds the r-th out-edge copy of every class with d > r
    # (a prefix of the window's d-desc class list).
    n_wr = np.zeros((NW, dmax), np.int64)
    for w in range(NW):
        for r in range(dmax):
            n_wr[w][r] = int(m[w][r + 1:].sum())
    xsec_rel = np.zeros((NW, dmax), np.int64)
    W_w = np.zeros(NW, np.int64)
    copy_list = []          # (w, dst_rel, n)
    for w in range(NW):
        o = 0
        for r in range(dmax):
            if n_wr[w][r] == 0:
                continue
            xsec_rel[w][r] = o
            copy_list.append((w, int(o), int(n_wr[w][r])))
            o += int(n_wr[w][r])
        W_w[w] = even(o)
    xw_off = np.r_[0, np.cumsum(W_w)]
    XW = int(xw_off[-1])

    # ---- layer maxima per (window, range) ----
    rw = cdiv(csh, NRANGE)
    widths = np.array([min(rw, csh - r * rw) for r in range(NRANGE)])
    Lmax = np.zeros((NW, NRANGE), np.int64)
    for c in cores_raw:
        r_v = c["c_v"] // rw
        np.maximum.at(Lmax, (c["w_e"], r_v), c["l_e"] + 1)
    Lmax = np.maximum(Lmax, 1)
    # enforce non-increasing in r (suffix max) for the prefix property
    for w in range(NW):
        for r in range(NRANGE - 2, -1, -1):
            Lmax[w][r] = max(Lmax[w][r], Lmax[w][r + 1])

    # ---- layer-major S layout per window ----
    secL_rel = np.zeros((NW, int(Lmax.max())), np.int64)
    W_l = np.zeros((NW, int(Lmax.max())), np.int64)
    S_w = np.zeros(NW, np.int64)
    for w in range(NW):
        o = 0
        for l in range(int(Lmax[w].max())):
            n_l = int((Lmax[w] > l).sum())
            wl = int(widths[:n_l].sum())
            secL_rel[w][l] = o
            W_l[w][l] = wl
            o += wl
        S_w[w] = even(o)
        assert S_w[w] <= MAXELEMS, f"S window {w} = {S_w[w]} > {MAXELEMS}"
    sm_off = np.r_[0, np.cumsum(S_w)]

    meta = dict(
        nc=nc, csh=csh, sh=sh, npad=npad, cf=cf, dmax=dmax, bpw=bpw,
        CLS=even(CLS), XW=XW, m=m, x0_off=x0_off, x0_woff=x0_woff,
        n_wr=n_wr, xsec_rel=xsec_rel, W_w=W_w, xw_off=xw_off,
        copy_list=copy_list, rw=rw, widths=widths, Lmax=Lmax,
        secL_rel=secL_rel, W_l=W_l, S_w=S_w, sm_off=sm_off,
    )
    per_core = [emit_core_arrays(c, meta) for c in cores_raw]
    return meta, per_core


def emit_core_arrays(c, meta):
    cf = meta["cf"]
    dmax = meta["dmax"]
    x0_off, x0_woff = meta["x0_off"], meta["x0_woff"]
    xsec_rel, W_w = meta["xsec_rel"], meta["W_w"]
    secL_rel = meta["secL_rel"]
    rw = meta["rw"]
    bpw = meta["bpw"]

    # class rank of each source within (p, w, d)
    ck = (c["usrc_p"] * NW + c["win_of_usrc"]) * (dmax + 1) + c["d_src"]
    cls_rank = _cumcount(ck)

    # ls1: gather state -> class layout
    ls1 = np.full((P, cf), -1, np.int16)
    tgt = x0_off[c["win_of_usrc"], c["d_src"]] + cls_rank
    assert tgt.max() < meta["CLS"]
    ls1[c["usrc_p"], c["usrc_q"]] = tgt.astype(np.int16)

    # window-relative class index per source, then per edge
    clsrel_src = (x0_off[c["win_of_usrc"], c["d_src"]]
                  - x0_woff[c["win_of_usrc"]] + cls_rank)
    si = c["src_inv"]
    xpos = xsec_rel[c["w_e"], c["r_in_src"]] + clsrel_src[si]

    # ls2: expanded X window -> transpose blocks
    ls2 = []
    for w in range(NW):
        a2 = np.full((P, int(W_w[w])), -1, np.int16)
        sel = c["w_e"] == w
        t2 = c["b_e"][sel] * P + c["j_e"][sel]
        assert len(t2) == 0 or t2.max() < bpw * P
        a2[c["p_s"][sel], xpos[sel]] = t2.astype(np.int16)
        ls2.append(a2)

    # ls3: transposed blocks -> layer-major S window
    r_v = c["c_v"] // rw
    ls3 = []
    for w in range(NW):
        arr = np.full((P, bpw * P), -1, np.int16)
        sel = c["w_e"] == w
        ipos = c["b_e"][sel] * P + c["p_s"][sel]
        t3 = secL_rel[w, c["l_e"][sel]] + c["c_v"][sel]
        assert len(t3) == 0 or t3.max() < meta["S_w"][w]
        arr[c["j_e"][sel], ipos] = t3.astype(np.int16)
        ls3.append(arr)

    return dict(ls1=ls1, ls2=ls2, ls3=ls3)


# ──────────────────────────────────────────────────────────────────────
# numpy emulation of one round (validation)
# ──────────────────────────────────────────────────────────────────────

def _emu_ls(data, idx, num_elems):
    Pp, n = idx.shape
    assert data.shape[0] == Pp and data.shape[1] >= n
    out = np.zeros((Pp, num_elems), data.dtype)
    for p in range(Pp):
        ii = idx[p].astype(np.int64)
        valid = ii >= 0
        assert len(np.unique(ii[valid])) == valid.sum(), "dup idx"
        out[p, ii[valid]] = data[p, :n][valid]
    return out


def emulate_round(w_full, meta, arrs, fdtype=np.float32):
    """w_full: [npad] pre-scaled state (device order). Returns u [SEGS,csh]
    = unscaled scatter-add for this core."""
    cf, csh = meta["cf"], meta["csh"]
    CLS, XW = meta["CLS"], meta["XW"]
    bpw = meta["bpw"]
    W_w, xw_off = meta["W_w"], meta["xw_off"]
    S_w, sm_off = meta["S_w"], meta["sm_off"]
    secL_rel, W_l, Lmax = meta["secL_rel"], meta["W_l"], meta["Lmax"]

    state = w_full.reshape(P, cf).astype(fdtype)
    x0 = _emu_ls(state, arrs["ls1"], CLS)

    X = np.zeros((P, XW), fdtype)
    for (w, dst, n) in meta["copy_list"]:
        X[:, xw_off[w] + dst: xw_off[w] + dst + n] = \
            x0[:, meta["x0_woff"][w]: meta["x0_woff"][w] + n]

    F = NW * bpw * P
    x2 = np.zeros((P, F), fdtype)
    for w in range(NW):
        o = _emu_ls(X[:, xw_off[w]:xw_off[w] + W_w[w]], arrs["ls2"][w],
                    bpw * P)
        x2[:, w * bpw * P:(w + 1) * bpw * P] = o

    xt = np.zeros((P, F), fdtype)
    for b in range(NW * bpw):
        xt[:, b * P:(b + 1) * P] = x2[:, b * P:(b + 1) * P].T

    y = np.zeros((NW, P, csh), fdtype)
    for w in range(NW):
        sm = _emu_ls(xt[:, w * bpw * P:(w + 1) * bpw * P], arrs["ls3"][w],
                     int(S_w[w]))
        for l in range(int(Lmax[w].max())):
            wl = int(W_l[w][l])
            y[w][:, :wl] += sm[:, int(secL_rel[w][l]):int(secL_rel[w][l]) + wl]

    u = np.zeros((SEGS, csh), fdtype)
    for w in range(NW):
        u += y[w].reshape(SEGS, R4, csh).sum(axis=1)
    return u


# ──────────────────────────────────────────────────────────────────────
# preprocess / inputs
# ──────────────────────────────────────────────────────────────────────

def next_pow2(x):
    p = 1
    while p < x:
        p *= 2
    return p


def preprocess(x, edge_index, batch, nc_count=8, n_graphs=128):
    n_nodes = x.shape[0]
    row = np.asarray(edge_index[0], np.int64)
    col = np.asarray(edge_index[1], np.int64)
    batch = np.asarray(batch, np.int64)

    csh, sh, npad, cf = build_layout(n_nodes, nc_count)
    deg = np.bincount(col, minlength=npad).astype(np.int64)
    flat, layout = relabel(deg, n_nodes, nc_count)
    re, ve = flat[row], flat[col]

    meta = per_core = None
    for bpw in (9, 10, 11):
        try:
            cores_raw = []
            for c in range(nc_count):
                mm = (ve // sh) == c
                cores_raw.append(build_core(c, re[mm], ve[mm], layout, bpw))
            meta, per_core = finalize_cores(cores_raw, layout, bpw)
            break
        except (AssertionError, RuntimeError):
            continue
    assert meta is not None, "routing build failed for all bpw"

    # device-order node arrays
    inv = np.empty(npad, np.int64)
    inv[flat] = np.arange(npad)
    deg_dev = deg[inv].astype(np.float32)
    batch_dev = np.full(npad, -1, np.int64)
    batch_dev[flat[:n_nodes]] = batch[:n_nodes]

    # ---- pooling structures ----
    g0 = np.zeros(nc_count, np.int64)
    ngl = np.zeros(nc_count, np.int64)
    wg_max = 0
    for c in range(nc_count):
        bd = batch_dev[c * sh:(c + 1) * sh]
        real = bd >= 0
        gmin, gmax = (int(bd[real].min()), int(bd[real].max())) \
            if real.any() else (0, 0)
        g0[c], ngl[c] = gmin, gmax - gmin + 1
        fin = np.arange(sh)
        s = fin // csh
        cnt = np.zeros((SEGS, int(ngl[c])), np.int64)
        np.add.at(cnt, (s[real], bd[real] - gmin), 1)
        wg_max = max(wg_max, int(cnt.max()))
    NGLP = int(ngl.max())
    WGP = next_pow2(wg_max)
    GPH = max(1, min(MAXELEMS // WGP, NGLP))
    NPH = cdiv(NGLP, GPH)
    PHALF = GPH * WGP
    assert NPH * GPH <= 32, f"pool pad {NPH * GPH} > 32"

    pool_idx = []
    gms = [np.zeros((P, P), np.float32) for _ in range(2)]
    for c in range(nc_count):
        bd = batch_dev[c * sh:(c + 1) * sh]
        fin = np.arange(sh)
        s, cc = fin // csh, fin % csh
        lg = bd - g0[c]
        arrs = []
        rank = np.zeros(sh, np.int64)
        real = bd >= 0
        key = s * 4096 + lg
        rank[real] = _cumcount(key[real])
        assert rank.max() < WGP
        for h in range(NPH):
            a = np.full((SEGS, csh), -1, np.int16)
            selh = real & (lg >= h * GPH) & (lg < (h + 1) * GPH)
            tgt = (lg[selh] - h * GPH) * WGP + rank[selh]
            assert len(tgt) == 0 or tgt.max() < PHALF
            a[s[selh], cc[selh]] = tgt.astype(np.int16)
            arrs.append(a)
        pool_idx.append(arrs)
        for li in range(NPH * GPH):
            g = g0[c] + li
            if li < int(ngl[c]) and g < n_graphs:
                fp = c * 32 + li
                gms[fp % 2][fp // 2, g] = 1.0

    pool_meta = dict(NGLP=NPH * GPH, NGH=GPH, WGP=WGP, NPH=NPH, PHALF=PHALF,
                     g0=g0)
    return meta, per_core, pool_meta, pool_idx, gms, flat, deg_dev, layout


def _xtpad(xsh):
    sh = xsh.shape[0]
    padsh = cdiv(sh, P) * P
    out = np.zeros((xsh.shape[1], padsh), np.float32)
    out[:, :sh] = xsh.T
    return out.astype(BF16)


def _degq(degsh):
    sh = len(degsh)
    nblk = cdiv(sh, P)
    pad = np.zeros(nblk * P, np.float32)
    pad[:sh] = degsh
    return np.ascontiguousarray(pad.reshape(nblk, P).T)


def make_inputs(meta, pool_meta, per_core, pool_idx, gms, flat, deg_dev,
                x, weights, n_max, n_graphs=128):
    csh, sh, npad, cf = meta["csh"], meta["sh"], meta["npad"], meta["cf"]
    nc_count = meta["nc"]
    n_nodes = x.shape[0]

    xdev = np.zeros((npad, x.shape[1]), np.float32)
    xdev[flat[:n_nodes]] = x
    in_maps = []
    for c in range(nc_count):
        im = dict(
            xT=_xtpad(xdev[c * sh:(c + 1) * sh]),
            degs=deg_dev[c * sh:(c + 1) * sh].reshape(SEGS, csh),
            degq=_degq(deg_dev[c * sh:(c + 1) * sh]),
            ls1=per_core[c]["ls1"],
            w1t=np.ascontiguousarray(weights["W1"].T),
            w2t=np.ascontiguousarray(weights["W2"].T),
            w3t=np.ascontiguousarray(weights["W3"].T),
            w4t=np.ascontiguousarray(weights["W4"].T),
            wl=np.ascontiguousarray(weights["Wl"]).reshape(64, 1),
            bl=np.asarray(weights["bl"], np.float32).reshape(1, 1),
            invn=np.asarray([[1.0 / np.float32(n_max)]], np.float32),
            selb=np.repeat(np.eye(SEGS, dtype=np.float32),
                           R4, axis=0).astype(BF16),
            identb=np.eye(P, dtype=np.float32).astype(BF16),
            ones16=np.ones((SEGS, 1), np.float32),
            gm0=gms[0], gm1=gms[1],
        )
        for k in range(1, 5):
            im[f"b{k}"] = np.asarray(weights[f"b{k}"], np.float32).reshape(64, 1)
        for w in range(NW):
            im[f"ls2_{w}"] = per_core[c]["ls2"][w]
            im[f"ls3_{w}"] = per_core[c]["ls3"][w]
        for h in range(pool_meta["NPH"]):
            im[f"pool_{h}"] = pool_idx[c][h]
        in_maps.append(im)
    return in_maps


def reference_numpy(x, edge_index, batch, weights, n_graphs=128):
    row = np.asarray(edge_index[0]); col = np.asarray(edge_index[1])
    N = x.shape[0]
    deg = np.bincount(col, minlength=N).astype(np.float64)
    dinv = np.where(deg > 0, deg ** -0.5, 0.0)
    norm = dinv[row] * dinv[col]
    h = x.astype(np.float64)
    for k in range(1, 5):
        W = weights[f"W{k}"]
        b = weights[f"b{k}"]
        hw = h @ W
        msg = norm[:, None] * hw[row]
        out = np.zeros((N, hw.shape[1]))
        np.add.at(out, col, msg)
        h = out + b
    sums = np.zeros((n_graphs, h.shape[1]))
    np.add.at(sums, batch, h)
    counts = np.bincount(batch, minlength=n_graphs)
    pooled = sums / counts.max()
    return (pooled @ weights["Wl"] + weights["bl"]).astype(np.float32)


# ──────────────────────────────────────────────────────────────────────
# device kernel
# ──────────────────────────────────────────────────────────────────────
from contextlib import ExitStack

import concourse.bass as bass
import concourse.tile as tile
from concourse import bacc, mybir

FP32 = mybir.dt.float32
BF16D = mybir.dt.bfloat16
I16 = mybir.dt.int16
AT = mybir.ActivationFunctionType
OP = mybir.AluOpType


def build_kernel(meta, pool_meta, n_graphs=128):
    csh, sh, npad, cf = meta["csh"], meta["sh"], meta["npad"], meta["cf"]
    CLS, XW = meta["CLS"], meta["XW"]
    bpw = meta["bpw"]
    W_w, xw_off = meta["W_w"], meta["xw_off"]
    S_w, sm_off = meta["S_w"], meta["sm_off"]
    secL_rel, W_l, Lmax = meta["secL_rel"], meta["W_l"], meta["Lmax"]
    NB = NW * bpw                       # total transpose blocks
    NGLP, WGP = pool_meta["NGLP"], pool_meta["WGP"]
    PHALF = pool_meta["PHALF"]
    NPH = pool_meta["NPH"]
    core_ids = list(range(meta["nc"]))
    cuts = [(0, min(512, csh))]
    if csh > 512:
        cuts.append((512, csh))

    nc = bacc.Bacc("TRN2", target_bir_lowering=False, debug=False,
                   num_devices=meta["nc"])

    def din(name, shape, dt=FP32):
        return nc.declare_dram_parameter(name, list(shape), dt, isOutput=False)

    # ---- inputs ----
    NBLK = cdiv(sh, P)                  # 128-node blocks for the s0 matvec
    lastw = sh - (NBLK - 1) * P
    xT_in = din("xT", [P, NBLK * P], BF16D)
    degs_in = din("degs", [SEGS, csh])
    degq_in = din("degq", [P, NBLK])
    ls1_in = din("ls1", [P, cf], I16)
    ls2_in = [din(f"ls2_{w}", [P, int(W_w[w])], I16) for w in range(NW)]
    ls3_in = [din(f"ls3_{w}", [P, bpw * P], I16) for w in range(NW)]
    pool_in = [din(f"pool_{h}", [SEGS, csh], I16) for h in range(NPH)]
    w1t_in = din("w1t", [64, 128])
    w2t_in = din("w2t", [64, 64])
    w3t_in = din("w3t", [64, 64])
    w4t_in = din("w4t", [64, 64])
    wl_in = din("wl", [64, 1])
    b_in = [din(f"b{k}", [64, 1]) for k in range(1, 5)]
    bl_in = din("bl", [1, 1])
    invn_in = din("invn", [1, 1])
    selb_in = din("selb", [P, SEGS], BF16D)
    identb_in = din("identb", [P, P], BF16D)
    ones16_in = din("ones16", [SEGS, 1])
    gm_in = [din(f"gm{i}", [P, P]) for i in range(2)]
    out_ext = nc.declare_dram_parameter("out", [n_graphs], FP32, isOutput=True)

    # ---- internal DRAM ----
    sh_dram = nc.dram_tensor("sh_dram", [sh], BF16D)
    full_dram = nc.dram_tensor("full_dram", [npad], BF16D, addr_space="Shared")
    part_dram = nc.dram_tensor("part_dram", [32], FP32)
    partall_dram = nc.dram_tensor("partall_dram", [256], FP32,
                                  addr_space="Shared")

    with tile.TileContext(nc) as tc:
        with ExitStack() as ctx:
            pool = ctx.enter_context(tc.tile_pool(name="p", bufs=1))
            tp = ctx.enter_context(tc.tile_pool(name="tp", bufs=2,
                                                space="PSUM"))
            up = ctx.enter_context(tc.tile_pool(name="up", bufs=2,
                                                space="PSUM"))

            # persistent tiles
            state = pool.tile([P, cf], BF16D)
            x0 = pool.tile([P, CLS], BF16D)
            X = pool.tile([P, XW], BF16D)
            x2 = pool.tile([P, NB * P], BF16D)
            xt = pool.tile([P, NB * P], BF16D)
            sm = pool.tile([P, int(sm_off[-1])], BF16D)
            dinvs = pool.tile([SEGS, csh], FP32)
            dinvs2 = pool.tile([SEGS, csh], FP32)
            bd = [pool.tile([SEGS, csh], FP32, name=f"bd{k}")
                  for k in range(3)]
            ts1 = pool.tile([SEGS, csh], FP32)
            wout = pool.tile([SEGS, csh], BF16D)
            s4 = pool.tile([SEGS, csh], FP32)
            degs = pool.tile([SEGS, csh], FP32)
            tmp16 = pool.tile([SEGS, csh], FP32)
            degq = pool.tile([P, NBLK], FP32)
            dinvq = pool.tile([P, NBLK], FP32)
            tmpq = pool.tile([P, NBLK], FP32)
            wout0 = pool.tile([P, NBLK], BF16D)
            selb = pool.tile([P, SEGS], BF16D)
            identb = pool.tile([P, P], BF16D)
            ones16 = pool.tile([SEGS, 1], FP32)
            gm = [pool.tile([P, P], FP32, name=f"gm{i}") for i in range(2)]
            ls1 = pool.tile([P, cf], I16)
            ls2 = [pool.tile([P, int(W_w[w])], I16, name=f"ls2t{w}")
                   for w in range(NW)]
            ls3 = [pool.tile([P, bpw * P], I16, name=f"ls3t{w}")
                   for w in range(NW)]
            plidx = [pool.tile([SEGS, csh], I16, name=f"plidx{h}")
                     for h in range(NPH)]
            poolsrc = pool.tile([SEGS, csh], BF16D)
            poolbuf = pool.tile([SEGS, NPH * PHALF], BF16D)
            poolf32 = pool.tile([SEGS, NPH * PHALF], FP32)
            p16 = pool.tile([SEGS, NGLP], FP32)
            part_sb = pool.tile([1, 32], FP32)
            partall = pool.tile([P, 2], FP32)
            outrow = pool.tile([1, n_graphs], FP32)
            wts = {
                "w1t": pool.tile([64, 128], FP32, name="w1t_t"),
                "w2t": pool.tile([64, 64], FP32, name="w2t_t"),
                "w3t": pool.tile([64, 64], FP32, name="w3t_t"),
                "w4t": pool.tile([64, 64], FP32, name="w4t_t"),
                "wl": pool.tile([64, 1], FP32, name="wl_t"),
            }
            bs = [pool.tile([64, 1], FP32, name=f"bs{k}") for k in range(4)]
            blt = pool.tile([1, 1], FP32)
            invn = pool.tile([1, 1], FP32)
            cvec = {
                "c3": pool.tile([64, 1], FP32, name="c3t"),
                "c2": pool.tile([64, 1], FP32, name="c2t"),
                "c1": pool.tile([64, 1], FP32, name="c1t"),
                "c0": pool.tile([128, 1], FP32, name="c0t"),
            }
            c0b = pool.tile([128, 1], BF16D)
            betas = pool.tile([1, 4], FP32)
            ones116 = pool.tile([1, 16], FP32)
            betas16 = pool.tile([SEGS, 4], FP32)

            # ---- loads ----
            nc.sync.dma_start(ls1[:], ls1_in[:])
            for w in range(NW):
                nc.sync.dma_start(ls2[w][:], ls2_in[w][:])
                nc.sync.dma_start(ls3[w][:], ls3_in[w][:])
            for h in range(NPH):
                nc.sync.dma_start(plidx[h][:], pool_in[h][:])
            nc.sync.dma_start(selb[:], selb_in[:])
            nc.sync.dma_start(identb[:], identb_in[:])
            nc.sync.dma_start(ones16[:], ones16_in[:])
            for i in range(2):
                nc.sync.dma_start(gm[i][:], gm_in[i][:])
            for k, t in wts.items():
                nc.sync.dma_start(t[:], {"w1t": w1t_in, "w2t": w2t_in,
                                         "w3t": w3t_in, "w4t": w4t_in,
                                         "wl": wl_in}[k][:])
            for k in range(4):
                nc.sync.dma_start(bs[k][:], b_in[k][:])
            nc.sync.dma_start(blt[:], bl_in[:])
            nc.sync.dma_start(invn[:], invn_in[:])
            nc.sync.dma_start(degs[:], degs_in[:])
            nc.sync.dma_start(degq[:], degq_in[:])

            # ---- dinv = rsqrt(deg + (deg==0)) * (deg>0) ----
            def make_dinv(dst, deg_t, tmp):
                nc.vector.tensor_scalar(tmp[:], deg_t[:], 0.0, None,
                                        OP.is_equal)
                nc.vector.tensor_tensor(tmp[:], tmp[:], deg_t[:], OP.add)
                nc.scalar.activation(tmp[:], tmp[:], AT.Sqrt)
                nc.vector.reciprocal(tmp[:], tmp[:])
                nc.vector.tensor_scalar(dst[:], deg_t[:], 0.0, None, OP.is_gt)
                nc.vector.tensor_tensor(dst[:], dst[:], tmp[:], OP.mult)

            make_dinv(dinvs, degs, tmp16)
            make_dinv(dinvq, degq, tmpq)
            nc.vector.tensor_tensor(dinvs2[:], dinvs[:], dinvs[:], OP.mult)

            # ---- c chain + betas ----
            pc = tp.tile([128, 4], FP32, tag="ops", bufs=1)
            nc.tensor.matmul(pc[0:64, 0:1], wts["w4t"][:], wts["wl"][:],
                             start=True, stop=True)
            nc.vector.tensor_copy(cvec["c3"][:], pc[0:64, 0:1])
            nc.tensor.matmul(pc[0:64, 1:2], wts["w3t"][:], cvec["c3"][:],
                             start=True, stop=True)
            nc.vector.tensor_copy(cvec["c2"][:], pc[0:64, 1:2])
            nc.tensor.matmul(pc[0:64, 2:3], wts["w2t"][:], cvec["c2"][:],
                             start=True, stop=True)
            nc.vector.tensor_copy(cvec["c1"][:], pc[0:64, 2:3])
            nc.tensor.matmul(pc[0:128, 3:4], wts["w1t"][:], cvec["c1"][:],
                             start=True, stop=True)
            nc.vector.tensor_copy(cvec["c0"][:], pc[0:128, 3:4])
            nc.vector.tensor_copy(c0b[:], cvec["c0"][:])
            pb = tp.tile([1, 4], FP32, tag="ops", bufs=1)
            for k, cn in enumerate(["c1", "c2", "c3"]):
                nc.tensor.matmul(pb[0:1, k:k + 1], bs[k][:], cvec[cn][:],
                                 start=True, stop=True)
            nc.tensor.matmul(pb[0:1, 3:4], bs[3][:], wts["wl"][:],
                             start=True, stop=True)
            nc.vector.tensor_copy(betas[:], pb[:])
            nc.vector.memset(ones116[:], 1.0)
            pbb = tp.tile([SEGS, 4], FP32, tag="ops", bufs=1)
            nc.tensor.matmul(pbb[:], ones116[:], betas[:], start=True,
                             stop=True)
            nc.vector.tensor_copy(betas16[:], pbb[:])
            # bd[k] = dinvs * beta_k   (k = 0..2 for rounds 0..2)
            for k in range(3):
                nc.vector.tensor_scalar(bd[k][:], dinvs[:],
                                        betas16[:, k:k + 1], None, OP.mult)

            # ---- s0 = x @ c0: stationary 128-node blocks, col k per block ----
            pu0 = up.tile([P, NBLK], FP32, tag="pu0", bufs=1, name="pu_s0")
            BPC = 25                     # blocks per xq chunk
            for q in range(cdiv(NBLK, BPC)):
                b0_, b1_ = q * BPC, min((q + 1) * BPC, NBLK)
                wq = (b1_ - b0_) * P
                xq = pool.tile([P, BPC * P], BF16D, tag="xq", bufs=2,
                               name=f"xq{q}")
                nc.sync.dma_start(xq[:, 0:wq],
                                  xT_in[:, b0_ * P:b0_ * P + wq])
                for b in range(b0_, b1_):
                    nc.tensor.matmul(
                        pu0[:, b:b + 1],
                        xq[:, (b - b0_) * P:(b - b0_ + 1) * P],
                        c0b[:], start=True, stop=True)
            nc.vector.tensor_tensor(wout0[:], pu0[:], dinvq[:], OP.mult)
            # transpose to block-major [NBLK, 128] for a contiguous DMA
            pt0 = tp.tile([P, P], BF16D, tag="ptr", name="pt_s0")
            nc.tensor.transpose(pt0[0:NBLK, :], wout0[:], identb[:])
            ws0 = pool.tile([P, P], BF16D)
            nc.vector.tensor_copy(ws0[0:NBLK, :], pt0[0:NBLK, :])

            # ---- rounds ----
            for rnd in range(4):
                if rnd == 0:
                    nc.sync.dma_start(
                        sh_dram[0:(NBLK - 1) * P].rearrange(
                            "(k m) -> k m", k=NBLK - 1),
                        ws0[0:NBLK - 1, :])
                    nc.sync.dma_start(
                        sh_dram[(NBLK - 1) * P:sh].rearrange(
                            "(a b) -> a b", a=1),
                        ws0[NBLK - 1:NBLK, 0:lastw])
                else:
                    nc.sync.dma_start(
                        sh_dram[:].rearrange("(a b) -> a b", a=SEGS), wout[:])
                nc.gpsimd.collective_compute(
                    "AllGather", OP.bypass, replica_groups=[core_ids],
                    ins=[sh_dram[:]], outs=[full_dram[:]],
                )
                nc.sync.dma_start(
                    state[:], full_dram[:].rearrange("(p c) -> p c", p=P))

                nc.gpsimd.local_scatter(
                    x0[:], state[:], ls1[:],
                    channels=P, num_elems=CLS, num_idxs=cf)
                for (w, dst, n) in meta["copy_list"]:
                    nc.vector.tensor_copy(
                        X[:, int(xw_off[w]) + dst:int(xw_off[w]) + dst + n],
                        x0[:, int(meta["x0_woff"][w]):
                           int(meta["x0_woff"][w]) + n])

                def emit_ls2(w):
                    nc.gpsimd.local_scatter(
                        x2[:, w * bpw * P:(w + 1) * bpw * P],
                        X[:, int(xw_off[w]):int(xw_off[w]) + int(W_w[w])],
                        ls2[w][:], channels=P, num_elems=bpw * P,
                        num_idxs=int(W_w[w]))

                def emit_transp(w):
                    blocks = list(range(w * bpw, (w + 1) * bpw))
                    for g0_ in range(0, len(blocks), 4):
                        grp = blocks[g0_:g0_ + 4]
                        pt = tp.tile([P, 4 * P], BF16D, tag="ptr",
                                     name=f"pt{rnd}_{w}_{g0_}")
                        for k, b in enumerate(grp):
                            nc.tensor.transpose(pt[:, k * P:(k + 1) * P],
                                                x2[:, b * P:(b + 1) * P],
                                                identb[:])
                        nc.vector.tensor_copy(
                            xt[:, grp[0] * P:(grp[-1] + 1) * P],
                            pt[:, 0:len(grp) * P])

                def emit_ls3(w):
                    nc.gpsimd.local_scatter(
                        sm[:, int(sm_off[w]):int(sm_off[w]) + int(S_w[w])],
                        xt[:, w * bpw * P:(w + 1) * bpw * P],
                        ls3[w][:], channels=P, num_elems=int(S_w[w]),
                        num_idxs=bpw * P)

                # keep gpsimd dense: all LS2s queued before the first LS3
                for w in range(NW):
                    emit_ls2(w)
                for w in range(NW):
                    emit_transp(w)
                    emit_ls3(w)

                # segment reduction: psum-accumulated sel matmuls over the
                # layer-major sections directly (fp32 accumulate in PSUM)
                pu = up.tile([SEGS, csh], FP32, tag="pu", name=f"pu{rnd}")
                mmlist = {a: [] for (a, b2) in cuts}
                for w in range(NW):
                    base = int(sm_off[w])
                    for l in range(int(Lmax[w].max())):
                        wl_ = int(W_l[w][l])
                        o = base + int(secL_rel[w][l])
                        for (a, b2) in cuts:
                            if wl_ <= a:
                                continue
                            mmlist[a].append((o, a, min(wl_, b2)))
                for a, lst in mmlist.items():
                    for i, (o, a_, e_) in enumerate(lst):
                        nc.tensor.matmul(pu[:, a_:e_], selb[:],
                                         sm[:, o + a_:o + e_],
                                         start=(i == 0),
                                         stop=(i == len(lst) - 1),
                                         skip_group_check=True)

                if rnd < 3:
                    nc.vector.tensor_tensor(ts1[:], pu[:], dinvs2[:], OP.mult)
                    nc.vector.tensor_tensor(wout[:], ts1[:], bd[rnd][:],
                                            OP.add)
                else:
                    nc.vector.tensor_tensor(ts1[:], pu[:], dinvs[:], OP.mult)
                    nc.vector.tensor_scalar(s4[:], ts1[:],
                                            betas16[:, 3:4], None, OP.add)

            # ---- pooling ----
            nc.vector.tensor_copy(poolsrc[:], s4[:])
            for h in range(NPH):
                nc.gpsimd.local_scatter(
                    poolbuf[:, h * PHALF:(h + 1) * PHALF],
                    poolsrc[:], plidx[h][:],
                    channels=SEGS, num_elems=PHALF, num_idxs=csh)
            nc.vector.tensor_copy(poolf32[:], poolbuf[:])
            wgp = WGP
            a = poolf32[:].rearrange("s (g t) -> s g t", t=WGP)
            while wgp > 1:
                hw = wgp // 2
                nc.vector.tensor_tensor(
                    a[:, :, 0:hw], a[:, :, 0:hw], a[:, :, hw:wgp], OP.add)
                wgp = hw
            nc.vector.tensor_copy(
                p16[:], a[:, :, 0:1].rearrange("s g t -> s (g t)"))
            pp = tp.tile([1, NGLP], FP32, tag="ops", bufs=1)
            nc.tensor.matmul(pp[:], ones16[:], p16[:], start=True, stop=True)
            nc.vector.memset(part_sb[:], 0.0)
            nc.vector.tensor_copy(part_sb[:, 0:NGLP], pp[:])
            nc.sync.dma_start(part_dram[:].rearrange("(a b) -> a b", a=1),
                              part_sb[:])
            nc.gpsimd.collective_compute(
                "AllGather", OP.bypass, replica_groups=[core_ids],
                ins=[part_dram[:]], outs=[partall_dram[:]],
            )
            nc.sync.dma_start(partall[:],
                              partall_dram[:].rearrange("(p c) -> p c", p=P))
            po_ = tp.tile([1, n_graphs], FP32, tag="ops", bufs=1)
            nc.tensor.matmul(po_[:], partall[:, 0:1], gm[0][:],
                             start=True, stop=False)
            nc.tensor.matmul(po_[:], partall[:, 1:2], gm[1][:],
                             start=False, stop=True)
            nc.vector.tensor_copy(outrow[:], po_[:])
            nc.vector.tensor_scalar(outrow[:], outrow[:], invn[0:1, 0:1],
                                    None, OP.mult)
            nc.vector.tensor_scalar(outrow[:], outrow[:], blt[0:1, 0:1],
                                    None, OP.add)
            nc.sync.dma_start(out_ext[:].rearrange("(a b) -> a b", a=1),
                              outrow[:])
    return nc


# ─── entry point ───

def kernel(x, edge_index, batch, W1, b1, W2, b2, W3, b3, W4, b4, Wl, bl):
    from concourse.bass_utils import run_bass_kernel_spmd

    x = np.asarray(x, np.float32)
    edge_index = np.asarray(edge_index)
    batch = np.asarray(batch)
    weights = dict(W1=np.asarray(W1, np.float32), W2=np.asarray(W2, np.float32),
                   W3=np.asarray(W3, np.float32), W4=np.asarray(W4, np.float32),
                   Wl=np.asarray(Wl, np.float32),
                   b1=np.asarray(b1, np.float32), b2=np.asarray(b2, np.float32),
                   b3=np.asarray(b3, np.float32), b4=np.asarray(b4, np.float32),
                   bl=np.asarray(bl, np.float32))
    n_graphs = 128

    meta, per_core, pool_meta, pool_idx, gms, flat, deg_dev, layout = \
        preprocess(x, edge_index, batch, 8, n_graphs)
    n_max = int(np.bincount(np.asarray(batch, np.int64),
                            minlength=n_graphs).max())
    in_maps = make_inputs(meta, pool_meta, per_core, pool_idx, gms, flat,
                          deg_dev, x, weights, n_max, n_graphs)
    nc = build_kernel(meta, pool_meta, n_graphs)
    nc.finalize()
    res = run_bass_kernel_spmd(nc, in_maps, core_ids=list(range(8)),
                               trace=False)
    return res.results[0]["out"].reshape(n_graphs, 1).astype(np.float32)
